# revision 28
# baseline (speedup 1.0000x reference)
"""GATv2 backbone (4 layers) on 8 Trainium2 NeuronCores.

Strategy (v4):
  * v3 + cross-layer phase fusion: each layer's normalize (P3) stages the
    NEXT layer's tables per window (transpose + xl/xr projection + DRAM
    bounce) and fires the per-band AllGathers early, removing the serial
    P3->P0 boundary. Gathers issue with single_packet=False (descriptors
    spread over all 16 SDMA engines); deeper tile buffering (gq bufs=3,
    per-quad chain bufs=4) keeps more quads in flight across the
    10-step cross-engine dependency chain.

Strategy (v3):
  * Nodes partitioned into 8 contiguous ranges (edge-balanced); edges owned
    by the dst core, grouped by 128-node dst windows. Windows are divided
    into SPLITS contiguous bands; each band's node table (<=32767 rows, so
    int16 gather indices work) is AllGathered separately, and the AllGathers
    are issued as soon as their band's xl shard is staged -- they pipeline
    with P0 and with edge-phase compute of earlier bands.
  * Gathers: nc.gpsimd.dma_gather, <=8 tiles (1024 rows) per op, one op per
    (window-group, band, chunk) -- ~130 ops/layer vs 850 indirect DMAs.
  * Hidden layers gather xl rows (256B); the final layer gathers h rows
    (256B) and computes xlf = h @ Wlf on-chip (4x less gather + AllGather
    traffic), with the numerator factored as (B_h^T @ gh) @ Wlf_h.
  * One-hot ST built bf16 in [e, d, t] layout (packed last dim -> DVE 2x
    mode); S = PE transpose of ST slices, PSUM->SBUF copies on the ACT
    engine. Edge-weight rank-1s and both linear biases fold into a single
    [ones; ew-rows] @ [bias; We-blocks] matmul per quad; numerator bias
    recovered via sum(alpha)=1. nmr|dnm share one PSUM accumulation matmul.
  * leaky_relu via Prelu (same ACT table set as Exp/Square -> no reloads).
"""

import contextlib
import os

import ml_dtypes
import numpy as np

from concourse import bass, bacc, mybir, tile
from concourse.bass_utils import run_bass_kernel_spmd
from concourse.masks import make_identity

P = 128
NCORES = 8
GMAX = 50
HEADS = 4
DHID = 128
CH = DHID // HEADS          # 32
DF = 512
NEG = 0.2
EPS = 1e-5
GW = 1                      # windows per gather group
MAXT = 8                    # tiles per dma_gather op (1024 idxs, HW limit)
NSPLIT = 4                  # table bands (pipelined AllGathers)
NQUEUES = 4                 # SWDGE queues for gather DMAs
USE_LRELU = os.environ.get("K_NO_PRELU", "") != "1"

F32 = mybir.dt.float32
BF = mybir.dt.bfloat16
I32 = mybir.dt.int32
I16 = mybir.dt.int16
AX = mybir.AxisListType
OP = mybir.AluOpType
AF = mybir.ActivationFunctionType


# ----------------------------------------------------------------------------
# Host preprocessing: graph partitioning + static schedule
# ----------------------------------------------------------------------------

def build_meta(edge_index, batch):
    N = batch.shape[0]
    E = edge_index.shape[1]
    src = np.asarray(edge_index[0], dtype=np.int64)
    dst = np.asarray(edge_index[1], dtype=np.int64)
    batch = np.asarray(batch, dtype=np.int64)

    deg = np.bincount(dst, minlength=N)
    cum = np.concatenate([[0], np.cumsum(deg)])

    bounds = [0]
    for c in range(1, NCORES):
        n = int(np.searchsorted(cum, c * E / NCORES))
        bounds.append(min(max(n, bounds[-1] + 1), N - (NCORES - c)))
    bounds.append(N)
    lo = np.array(bounds[:-1])
    hi = np.array(bounds[1:])

    NW = int(max((hi - lo + P - 1) // P))
    NPAD = NW * P
    S = min(NSPLIT, NW)
    swin = [a for a in np.array_split(np.arange(NW), S)]
    sbase = np.array([int(a[0]) for a in swin])
    scount = np.array([len(a) for a in swin])
    TBLROWS = NCORES * scount * P
    assert (TBLROWS <= 32767).all(), TBLROWS
    split_id = np.zeros(NW, np.int64)
    for si, a in enumerate(swin):
        split_id[a] = si

    core_of = np.repeat(np.arange(NCORES), (hi - lo))
    off = np.arange(N) - lo[core_of]
    wof = off // P                       # window-within-core (may pad-overflow
    wof = np.minimum(wof, NW - 1)        # never: off < NPAD)
    nsp = split_id[wof]
    nrow = core_of * scount[nsp] * P + (off - sbase[nsp] * P)

    order = np.argsort(dst, kind="stable")

    cnt = np.zeros((S, NCORES, NW), np.int64)
    lists = {}
    for c in range(NCORES):
        for w in range(NW):
            a = lo[c] + w * P
            b = min(a + P, hi[c])
            if a >= b:
                lists[(c, w)] = [np.empty(0, np.int64)] * S
                continue
            ids = order[int(cum[a]):int(cum[b])]
            sp = nsp[src[ids]]
            per = []
            for si in range(S):
                idsS = ids[sp == si]
                idsS = idsS[np.argsort(nrow[src[idsS]], kind="stable")]
                per.append(idsS)
                cnt[si, c, w] = len(idsS)
            lists[(c, w)] = per

    TS = ((cnt.max(axis=1) + P - 1) // P).astype(np.int64)    # [S, NW]
    for w in range(NW):
        if TS[:, w].sum() == 0:
            TS[0, w] = 1
    run_off = np.zeros((S, NW), np.int64)
    for si in range(1, S):
        run_off[si] = run_off[si - 1] + TS[si - 1]
    Tw = TS.sum(axis=0)
    woff = np.concatenate([[0], np.cumsum(Tw)]).astype(np.int64)
    TT = int(woff[-1])

    dsti = np.full((NCORES, P, TT), -1, np.int64)
    ewsl = np.full((NCORES, TT * P), -1, np.int64)
    srow = np.zeros((NCORES, TT * P), np.int64)
    for c in range(NCORES):
        for w in range(NW):
            for si in range(S):
                ids_h = lists[(c, w)][si]
                n = len(ids_h)
                if n == 0:
                    continue
                tb = int(woff[w] + run_off[si, w])
                slot = np.arange(n)
                tt = tb + slot // P
                pp = slot % P
                dsti[c, pp, tt] = dst[ids_h] - (lo[c] + w * P)
                flat = tt * P + pp
                ewsl[c, flat] = ids_h
                srow[c, flat] = nrow[src[ids_h]]

    # per-window quad schedule (band-major)
    nqS = ((TS + 3) // 4).astype(np.int64)            # [S, NW]
    nqcum = np.zeros((S, NW), np.int64)
    for si in range(1, S):
        nqcum[si] = nqcum[si - 1] + nqS[si - 1]
    nquad = nqS.sum(axis=0)
    NQBMAX = int(nqS.max())
    qoff = np.concatenate([[0], np.cumsum(nquad)]).astype(np.int64)
    NQTOT = int(qoff[-1])
    NQMAX = int(nquad.max())

    # gather-op schedule (static, identical across cores)
    groups = []
    icols = 0
    for g0 in range(0, NW, GW):
        ws = list(range(g0, min(g0 + GW, NW)))
        smap = [dict() for _ in range(S)]
        tgb = []
        ops = []
        for si in range(S):
            run = 0
            for w in ws:
                smap[si][w] = run
                run += int(TS[si, w])
            tgb.append(run)
            tiles = []
            for w in ws:
                for t in range(int(TS[si, w])):
                    gt = int(woff[w] + run_off[si, w]) + t
                    tiles.append((gt, smap[si][w] + t))
            for i in range(0, len(tiles), MAXT):
                ch = tiles[i:i + MAXT]
                ops.append(dict(split=si, coff=icols, nt=len(ch),
                                gq0=ch[0][1], gtiles=[x[0] for x in ch]))
                icols += len(ch) * 8
        groups.append(dict(ws=ws, tgb=tgb, smap=smap, ops=ops))
    ICOLS = icols
    TGMAX = max(max(g["tgb"]) for g in groups)
    TGFULL = max(sum(g["tgb"]) for g in groups)
    TWMAX = int(Tw.max())
    fnz = np.array([min(si for si in range(S) if TS[si, w] > 0)
                    for w in range(NW)])
    lnz = np.array([max(si for si in range(S) if TS[si, w] > 0)
                    for w in range(NW)])

    idxm = np.zeros((NCORES, 16, ICOLS), np.int16)
    for c in range(NCORES):
        for g in groups:
            for op in g["ops"]:
                nt = op["nt"]
                vals = np.zeros(nt * P, np.int64)
                for j, gt in enumerate(op["gtiles"]):
                    vals[j * P:(j + 1) * P] = srow[c, gt * P:(gt + 1) * P]
                idxm[c, :, op["coff"]:op["coff"] + nt * 8] = (
                    vals.reshape(nt * 8, 16).T.astype(np.int16))

    gmat = np.zeros((NCORES, P, NW * GMAX), np.float32)
    gmatT = np.zeros((NCORES, GMAX, NW * P), np.float32)
    for c in range(NCORES):
        nreal = int(hi[c] - lo[c])
        g = batch[lo[c]:hi[c]]
        r = np.arange(nreal)
        gmat[c, r % P, (r // P) * GMAX + g] = 1.0
        gmatT[c, g, (r // P) * P + (r % P)] = 1.0

    cntg = np.bincount(batch, minlength=GMAX).astype(np.float32)
    invd = (1.0 / (np.maximum(cntg, 1.0) * DHID)).reshape(1, GMAX)

    return dict(N=N, E=E, NW=NW, NPAD=NPAD, S=S, swin=swin, sbase=sbase,
                scount=scount, TBLROWS=TBLROWS, TT=TT, ICOLS=ICOLS,
                TGMAX=TGMAX, TWMAX=TWMAX, TS=TS, run_off=run_off, Tw=Tw,
                woff=woff, lo=lo, hi=hi, nqS=nqS, nqcum=nqcum, nquad=nquad,
                qoff=qoff, NQTOT=NQTOT, NQMAX=NQMAX, NQBMAX=NQBMAX,
                TGFULL=TGFULL,
                groups=groups, fnz=fnz, lnz=lnz,
                dsti=dsti, ewsl=ewsl, idxm=idxm, gmat=gmat, gmatT=gmatT,
                invd=invd)


# ----------------------------------------------------------------------------
# Bass program
# ----------------------------------------------------------------------------

def build_program(meta):
    NW, NPAD, TT = meta["NW"], meta["NPAD"], meta["TT"]
    S, sbase, scount = meta["S"], meta["sbase"], meta["scount"]
    TBLROWS, ICOLS = meta["TBLROWS"], meta["ICOLS"]
    TGMAX, TWMAX = meta["TGMAX"], meta["TWMAX"]
    TS, run_off, woff = meta["TS"], meta["run_off"], meta["woff"]
    nqS, nqcum, nquad, qoff = (meta["nqS"], meta["nqcum"], meta["nquad"],
                               meta["qoff"])
    NQTOT, NQMAX = meta["NQTOT"], meta["NQMAX"]
    NQBMAX = meta["NQBMAX"]
    TGFULL = meta["TGFULL"]
    groups = meta["groups"]
    swin = meta["swin"]
    fnz, lnz = meta["fnz"], meta["lnz"]

    nc = bacc.Bacc("TRN2", target_bir_lowering=False, debug=False,
                   enable_asserts=False, num_devices=NCORES,
                   num_swdge_queues=1)

    h0s_d = nc.dram_tensor("h0s", [P, NW * P], BF, kind="ExternalInput")
    rs_d = nc.dram_tensor("rs", [P, NW * P], BF, kind="ExternalInput")
    dsti_d = nc.dram_tensor("dsti", [P, TT], BF, kind="ExternalInput")
    idx_d = nc.dram_tensor("idx", [P, ICOLS], I16, kind="ExternalInput")
    ew5_d = nc.dram_tensor("ew5", [5, NQTOT * P], BF, kind="ExternalInput")
    ew2_d = nc.dram_tensor("ew2", [2, TT * P], BF, kind="ExternalInput")
    gmat_d = nc.dram_tensor("gmat", [P, NW * GMAX], BF, kind="ExternalInput")
    gmatT_d = nc.dram_tensor("gmatT", [GMAX, NW * P], BF, kind="ExternalInput")
    invd_d = nc.dram_tensor("invd", [1, GMAX], F32, kind="ExternalInput")
    cpk_d = nc.dram_tensor("cpk", [P, 9 * P], F32, kind="ExternalInput")
    apk_d = nc.dram_tensor("apk", [P, 3 * P], BF, kind="ExternalInput")
    wpk_d = nc.dram_tensor("wpk", [P, 6 * P + 2 * DF], BF, kind="ExternalInput")
    attf_d = nc.dram_tensor("attf", [P, DF], BF, kind="ExternalInput")
    fpk_d = nc.dram_tensor("fpk", [P, P], F32, kind="ExternalInput")
    webb_d = nc.dram_tensor("webb", [5, 12 * P], BF, kind="ExternalInput")
    fwebb_d = nc.dram_tensor("fwebb", [2, DF], BF, kind="ExternalInput")
    out_d = nc.dram_tensor("out", [NPAD, P], F32, kind="ExternalOutput")

    with tile.TileContext(nc) as tc, contextlib.ExitStack() as ctx:
        dram = ctx.enter_context(tc.tile_pool(name="dram", bufs=1, space="DRAM"))
        cst = ctx.enter_context(tc.tile_pool(name="cst", bufs=1))
        per = ctx.enter_context(tc.tile_pool(name="per", bufs=1))
        wsp = ctx.enter_context(tc.tile_pool(name="wsp", bufs=2))
        gpo = ctx.enter_context(tc.tile_pool(name="gpo", bufs=2))

        xl_b = dram.tile([NPAD, P], BF)
        tbl = [nc.dram_tensor(f"tbl{si}", [int(TBLROWS[si]), P], BF,
                              kind="Internal", addr_space="Shared")
               for si in range(S)]
        st_b = dram.tile([2, GMAX], F32)
        st_o = dram.tile([2, GMAX], F32)
        cgroups = [list(range(NCORES))]

        # --- constants / residents ---
        ident = cst.tile([P, P], F32)
        make_identity(nc, ident[:])
        identb = cst.tile([P, P], BF)
        nc.vector.tensor_copy(out=identb[:], in_=ident[:])
        iota_row = cst.tile([P, P], I32)
        nc.gpsimd.iota(iota_row[:], pattern=[[1, P]], base=0,
                       channel_multiplier=0)
        iota_rowb = cst.tile([P, P], BF)
        nc.vector.tensor_copy(out=iota_rowb[:], in_=iota_row[:])
        iota3 = cst.tile([P, P, TWMAX], BF)
        nc.vector.tensor_copy(
            out=iota3[:],
            in_=iota_rowb[:, :, None].to_broadcast([P, P, TWMAX]))
        epsc = cst.tile([P, 1], F32)
        nc.vector.memset(epsc[:], EPS)
        invd = cst.tile([1, GMAX], F32)
        nc.sync.dma_start(out=invd[:], in_=invd_d[:, :])
        wpk_s = cst.tile([P, 6 * P + 2 * DF], BF)
        nc.sync.dma_start(out=wpk_s[:], in_=wpk_d[:, :])
        dsti_s = cst.tile([P, TT], BF)
        nc.sync.dma_start(out=dsti_s[:], in_=dsti_d[:, :])
        idx_s = cst.tile([P, ICOLS], I16)
        nc.sync.dma_start(out=idx_s[:], in_=idx_d[:, :])
        gmat_s = cst.tile([P, NW * GMAX], BF)
        nc.sync.dma_start(out=gmat_s[:], in_=gmat_d[:, :])
        gmatT_s = cst.tile([GMAX, NW * P], BF)
        nc.sync.dma_start(out=gmatT_s[:], in_=gmatT_d[:, :])
        cpk_s = cst.tile([P, 9 * P], F32)
        nc.sync.dma_start(out=cpk_s[:], in_=cpk_d[:, :])
        apk_s = cst.tile([P, 3 * P], BF)
        nc.sync.dma_start(out=apk_s[:], in_=apk_d[:, :])
        attf_s = cst.tile([P, DF], BF)
        nc.sync.dma_start(out=attf_s[:], in_=attf_d[:, :])
        fpk_s = cst.tile([P, P], F32)
        nc.sync.dma_start(out=fpk_s[:], in_=fpk_d[:, :])
        webb_s = cst.tile([5, 12 * P], BF)
        nc.sync.dma_start(out=webb_s[:], in_=webb_d[:, :])
        fwebb_s = cst.tile([2, DF], BF)
        nc.sync.dma_start(out=fwebb_s[:], in_=fwebb_d[:, :])

        h_a = per.tile([P, NW, P], BF, tag="h_a")
        hT = per.tile([P, NW, P], BF, tag="hT")
        nc.sync.dma_start(out=h_a[:, :, :],
                          in_=h0s_d[:, :].rearrange("p (w f) -> p w f", w=NW))

        def leaky(dst_ap, src_ap, shape):
            if USE_LRELU:
                # Prelu == leaky relu with param alpha; unlike Lrelu it is in
                # the same ACT table set as Exp/Square -> no table reloads.
                nc.scalar.activation(out=dst_ap, in_=src_ap, func=AF.Prelu,
                                     alpha=NEG)
            else:
                r = wsp.tile(shape, F32, tag="lrtmp", bufs=1, name="lr")
                rr = r[tuple(slice(0, s) for s in dst_ap.shape)]
                nc.scalar.activation(out=rr, in_=src_ap, func=AF.Relu,
                                     scale=-(1.0 - NEG))
                nc.vector.tensor_tensor(out=dst_ap, in0=src_ap, in1=rr,
                                        op=OP.add)

        split_last = {int(a[-1]): si for si, a in enumerate(swin)}
        SGW = 2                 # staging chunk (windows per xl_b DMA)
        breaks = set(range(SGW - 1, NW, SGW)) | set(split_last) | {NW - 1}

        def emit_ag(si):
            a = int(sbase[si]) * P
            b = a + int(scount[si]) * P
            nc.gpsimd.collective_compute(
                "AllGather", OP.bypass, replica_groups=cgroups,
                ins=[xl_b[a:b, :].opt()], outs=[tbl[si][:, :].opt()])

        qctr = [0]

        def grp_gathers(g, band=None):
            if band is not None:
                gq = gpo.tile([P, TGMAX, P], BF, tag="gq", name="gq",
                              bufs=5)
            else:
                gq = gpo.tile([P, TGFULL, P], BF, tag="gqf", name="gqf",
                              bufs=3)
            boff = np.concatenate([[0], np.cumsum(g["tgb"])]).astype(int)
            for op in g["ops"]:
                if band is not None and op["split"] != band:
                    continue
                g0 = op["gq0"] + (0 if band is not None
                                  else int(boff[op["split"]]))
                nc.gpsimd.dma_gather(
                    gq[:, g0:g0 + op["nt"], :],
                    tbl[op["split"]][:, :],
                    idx_s[:, op["coff"]:op["coff"] + op["nt"] * 8],
                    op["nt"] * P, op["nt"] * P, P,
                    single_packet=False)
            return gq, boff

        def build_st(w, t0, Tn):
            """ST2[e, d, t] one-hot (bf16, packed last dim -> DVE 2x)."""
            ST = wsp.tile([P, P, TWMAX], BF, tag="ST", name="ST")
            nc.vector.tensor_tensor(
                out=ST[:, :, :Tn],
                in0=iota3[:, :, :Tn],
                in1=dsti_s[:, None, t0:t0 + Tn]
                    .to_broadcast([P, P, Tn]),
                op=OP.is_equal)
            return ST

        # ------------------------------------------------------------------
        def make_stager(li):
            """Staging for layer li's tables: transpose h, project (hidden)
            or copy (final), bounce to DRAM, fire per-band AllGathers.
            Called per window, fused into the previous layer's P3."""
            st = dict(run=False, w0=0, xsg=None)
            if li < 3:
                st["xr_all"] = wsp.tile([P, NW, P], BF, tag="xra", bufs=1,
                                        name="xra")
                wl = wpk_s[:, li * P:(li + 1) * P]
                wr = wpk_s[:, (3 + li) * P:(4 + li) * P]

            def stage(w, ps):
                nc.sync.dma_start(out=hT[:, w, :], in_=h_a[:, w, :],
                                  transpose=True)
                if not st["run"]:
                    st["run"] = True
                    st["w0"] = w
                    if li < 3:
                        st["xsg"] = wsp.tile([P, SGW, P], BF, tag="xsg",
                                             name="xsg")
                w0 = st["w0"]
                if li < 3:
                    xp = ps.tile([P, P], F32, tag="px", bufs=1, name="px")
                    nc.tensor.matmul(out=xp[:], lhsT=hT[:, w, :], rhs=wl,
                                     start=True, stop=True)
                    nc.scalar.activation(out=st["xsg"][:, w - w0, :],
                                         in_=xp[:], func=AF.Identity)
                    xrp = ps.tile([P, P], F32, tag="px", bufs=1, name="xrp")
                    nc.tensor.matmul(out=xrp[:], lhsT=hT[:, w, :], rhs=wr,
                                     start=True, stop=True)
                    nc.scalar.activation(out=st["xr_all"][:, w, :],
                                         in_=xrp[:], func=AF.Identity)
                    if w in breaks:
                        nc.sync.dma_start(
                            out=xl_b[w0 * P:(w + 1) * P, :].rearrange(
                                "(w p) f -> p w f", p=P),
                            in_=st["xsg"][:, :w - w0 + 1, :])
                        st["run"] = False
                else:
                    if w in breaks:
                        nc.sync.dma_start(
                            out=xl_b[w0 * P:(w + 1) * P, :].rearrange(
                                "(w p) f -> p w f", p=P),
                            in_=h_a[:, w0:w + 1, :])
                        st["run"] = False
                if w in split_last:
                    emit_ag(split_last[w])
            return st, stage

        def hidden_layer(li, add_resid, xr_all, next_li):
            attr = apk_s[:, li * P:(li + 1) * P]
            lnw = cpk_s[:, li * P:(li + 1) * P]
            lnb = cpk_s[:, (3 + li) * P:(4 + li) * P]
            bia = cpk_s[:, (6 + li) * P:(7 + li) * P]

            with tc.tile_pool(name=f"ps{li}", bufs=1, space="PSUM") as ps:
                # P2: edge pipeline, band-major so AllGather si+1 overlaps
                # band-si compute; per-window numerators accumulate in SBUF.
                nd_all = wsp.tile([P, NW, P + HEADS], BF, tag="nda",
                                  bufs=1, name="nda")
                for band in range(S):
                    for g in groups:
                        if all(TS[band, w] == 0 for w in g["ws"]):
                            continue
                        gq, _ = grp_gathers(g, band)
                        gqv = gq[:].rearrange("p t (c h) -> p t c h",
                                              h=HEADS, c=CH)
                        for w in g["ws"]:
                            Th = int(TS[band, w])
                            if Th == 0:
                                continue
                            t0g = int(woff[w] + run_off[band, w])
                            ST = build_st(w, t0g, Th)
                            nqb = (Th + 3) // 4
                            qb0 = int(qoff[w] + nqcum[band, w])
                            ews5 = wsp.tile([5, NQBMAX * P], BF, tag="ews",
                                            bufs=4, name="ews")
                            nc.sync.dma_start(
                                out=ews5[0:5, :nqb * P],
                                in_=ew5_d[0:5, qb0 * P:(qb0 + nqb) * P])
                            nd = ps.tile([P, P + HEADS], F32, tag="nd",
                                         name="nd")
                            gq0 = g["smap"][band][w]

                            def emit_nd(q0, Q, mmw):
                                for t in range(Q):
                                    nc.tensor.matmul(
                                        out=nd[:], lhsT=ST[:, :, q0 + t],
                                        rhs=mmw[:, t, :],
                                        start=(q0 + t == 0),
                                        stop=(q0 + t == Th - 1))

                            pend = None
                            for q0 in range(0, Th, 4):
                                Q = min(4, Th - q0)
                                qq = q0 // 4
                                Ssb = wsp.tile([P, 4, P], BF, tag="ssb",
                                               bufs=5, name="ssb")
                                sp = ps.tile([P, 4, P], BF, tag="pt",
                                             bufs=2, name="sp")
                                for t in range(Q):
                                    nc.tensor.transpose(
                                        out=sp[:, t, :],
                                        in_=ST[:, :, q0 + t],
                                        identity=identb[:])
                                nc.scalar.activation(out=Ssb[:, :Q, :],
                                                     in_=sp[:, :Q, :],
                                                     func=AF.Identity)
                                ep = ps.tile([P, 4 * P], F32, tag="ep",
                                             bufs=3, name="ep")
                                # ef + biases first (host data, always ready)
                                # and the gather-dependent copy last, so the
                                # in-order PE queue head never parks on a
                                # not-yet-landed DMA.
                                nc.tensor.matmul(
                                    out=ep[:, :Q * P],
                                    lhsT=ews5[0:Q + 1, qq * P:(qq + 1) * P],
                                    rhs=webb_s[0:Q + 1,
                                               li * 4 * P:li * 4 * P + Q * P],
                                    start=True, stop=False)
                                for t in range(Q):
                                    blk = ep[:, t * P:(t + 1) * P]
                                    nc.tensor.matmul(out=blk,
                                                     lhsT=Ssb[:, t, :],
                                                     rhs=xr_all[:, w, :],
                                                     start=False, stop=False)
                                nc.tensor.matmul(
                                    out=ep[:, :Q * P], lhsT=identb[:],
                                    rhs=gq[:, gq0 + q0:gq0 + q0 + Q, :],
                                    start=False, stop=True)
                                ea = wsp.tile([P, 4 * P], BF, tag="ea", bufs=5,
                                              name="ea")
                                leaky(ea[:, :Q * P], ep[:, :Q * P], [P, 4 * P])
                                lg = wsp.tile([P, 4 * P], BF, tag="lg", bufs=5,
                                              name="lg")
                                nc.vector.tensor_tensor(
                                    out=lg[:, :Q * P], in0=ea[:, :Q * P],
                                    in1=attr[:, None, :].to_broadcast(
                                        [P, Q, P]),
                                    op=OP.mult)
                                lgr = wsp.tile([P, 4 * HEADS], F32, tag="lgr",
                                               bufs=4, name="lgr")
                                nc.vector.tensor_reduce(
                                    out=lgr[:, :Q * HEADS].rearrange(
                                        "p (t h) -> p t h", h=HEADS),
                                    in_=lg[:].rearrange(
                                        "p (t c h) -> p t h c", h=HEADS,
                                        c=CH)[:, :Q, :, :],
                                    axis=AX.X, op=OP.add)
                                mmw = wsp.tile([P, 4, P + HEADS], BF,
                                               tag="mm", bufs=4, name="mm")
                                nc.scalar.activation(
                                    out=mmw[:, :Q, P:P + HEADS],
                                    in_=lgr[:, :Q * HEADS].rearrange(
                                        "p (t h) -> p t h", h=HEADS),
                                    func=AF.Exp)
                                nc.vector.tensor_tensor(
                                    out=mmw[:, :Q, 0:P].rearrange(
                                        "p q (c h) -> p q c h", h=HEADS,
                                        c=CH),
                                    in0=gqv[:, gq0 + q0:gq0 + q0 + Q, :, :],
                                    in1=mmw[:, :Q, None, P:P + HEADS]
                                        .to_broadcast([P, Q, CH, HEADS]),
                                    op=OP.mult)
                                if pend is not None:
                                    emit_nd(*pend)
                                pend = (q0, Q, mmw)
                            if pend is not None:
                                emit_nd(*pend)
                            if band == int(fnz[w]):
                                nc.scalar.activation(out=nd_all[:, w, :],
                                                     in_=nd[:],
                                                     func=AF.Identity)
                            else:
                                nc.vector.tensor_tensor(
                                    out=nd_all[:, w, :], in0=nd_all[:, w, :],
                                    in1=nd[:], op=OP.add)

                # window flush + LN stats
                stp = ps.tile([2, GMAX], F32, tag="stats", name="stp")
                for w in range(NW):
                    rd = wsp.tile([P, HEADS], F32, tag="rd", name="rd")
                    nc.vector.tensor_scalar(out=rd[:],
                                            in0=nd_all[:, w, P:P + HEADS],
                                            scalar1=1e-16, scalar2=None,
                                            op0=OP.add)
                    nc.vector.reciprocal(out=rd[:], in_=rd[:])
                    oT = wsp.tile([P, HEADS, CH], F32, tag="oT", name="oT")
                    nc.vector.tensor_tensor(
                        out=oT[:],
                        in0=nd_all[:, w, :P].rearrange("p (c h) -> p h c",
                                                       h=HEADS, c=CH),
                        in1=rd[:, :, None].to_broadcast([P, HEADS, CH]),
                        op=OP.mult)
                    nc.vector.tensor_tensor(
                        out=h_a[:, w, :],
                        in0=oT[:].rearrange("p h c -> p (h c)"),
                        in1=bia, op=OP.add)
                    s12 = wsp.tile([P, 2], F32, tag="s12", name="s12")
                    nc.vector.tensor_reduce(out=s12[:, 0:1],
                                            in_=h_a[:, w, :],
                                            axis=AX.X, op=OP.add)
                    sqj = wsp.tile([P, P], BF, tag="sqj", name="sqj")
                    nc.scalar.activation(out=sqj[:], in_=h_a[:, w, :],
                                         func=AF.Square,
                                         accum_out=s12[:, 1:2])
                    s12b = wsp.tile([P, 2], BF, tag="s12b", name="s12b")
                    nc.vector.tensor_copy(out=s12b[:], in_=s12[:])
                    nc.tensor.matmul(
                        out=stp[:, :], lhsT=s12b[:],
                        rhs=gmat_s[:, w * GMAX:(w + 1) * GMAX],
                        start=(w == 0), stop=(w == NW - 1))

                # P3: stats -> mean/rstd -> normalize + elu
                sts = wsp.tile([2, GMAX], F32, tag="sts", name="sts")
                nc.vector.tensor_copy(out=sts[:], in_=stp[:])
                nc.sync.dma_start(out=st_b[:, :], in_=sts[:])
                nc.gpsimd.collective_compute(
                    "AllReduce", OP.add, replica_groups=cgroups,
                    ins=[st_b.opt()], outs=[st_o.opt()])
                stg1 = wsp.tile([1, GMAX], F32, tag="stg1", name="stg1")
                nc.sync.dma_start(out=stg1[:], in_=st_o[0:1, :])
                stg2 = wsp.tile([1, GMAX], F32, tag="stg2", name="stg2")
                nc.sync.dma_start(out=stg2[:], in_=st_o[1:2, :])
                mean = wsp.tile([1, GMAX], F32, tag="mean", name="mean")
                nc.vector.tensor_tensor(out=mean[:], in0=stg1[:],
                                        in1=invd[:], op=OP.mult)
                ex2 = wsp.tile([1, GMAX], F32, tag="ex2", name="ex2")
                nc.vector.tensor_tensor(out=ex2[:], in0=stg2[:],
                                        in1=invd[:], op=OP.mult)
                msq = wsp.tile([1, GMAX], F32, tag="msq", name="msq")
                nc.scalar.activation(out=msq[:], in_=mean[:], func=AF.Square)
                var = wsp.tile([1, GMAX], F32, tag="var", name="var")
                nc.vector.tensor_tensor(out=var[:], in0=ex2[:], in1=msq[:],
                                        op=OP.subtract)
                sd = wsp.tile([1, GMAX], F32, tag="sd", name="sd")
                nc.scalar.activation(out=sd[:], in_=var[:], func=AF.Sqrt,
                                     bias=epsc[0:1, 0:1])
                rstd = wsp.tile([1, GMAX], F32, tag="rstd", name="rstd")
                nc.vector.reciprocal(out=rstd[:], in_=sd[:])
                nmr2 = wsp.tile([1, GMAX], F32, tag="nmr2", name="nm2")
                nc.vector.tensor_tensor(out=nmr2[:], in0=mean[:], in1=rstd[:],
                                        op=OP.mult)
                nc.vector.tensor_scalar(out=nmr2[:], in0=nmr2[:], scalar1=-1.0,
                                        scalar2=None, op0=OP.mult)
                t1 = ps.tile([P, P], F32, tag="px", bufs=1, name="t1")
                nc.tensor.transpose(out=t1[0:GMAX, 0:1], in_=nmr2[:],
                                    identity=ident[0:1, 0:1])
                t2 = ps.tile([P, P], F32, tag="px", bufs=1, name="t2")
                nc.tensor.transpose(out=t2[0:GMAX, 0:1], in_=rstd[:],
                                    identity=ident[0:1, 0:1])
                nrcol = wsp.tile([GMAX, 2], BF, tag="nrcol", name="nrc")
                nc.vector.tensor_copy(out=nrcol[:, 0:1], in_=t1[0:GMAX, 0:1])
                nc.vector.tensor_copy(out=nrcol[:, 1:2], in_=t2[0:GMAX, 0:1])

                st_n, stage_n = make_stager(next_li)
                for w in range(NW):
                    mw = ps.tile([P, P], F32, tag="px", bufs=1, name="mw")
                    nc.tensor.matmul(out=mw[:, 0:2],
                                     lhsT=gmatT_s[:, w * P:(w + 1) * P],
                                     rhs=nrcol[:], start=True, stop=True)
                    mws = wsp.tile([P, 2], F32, tag="mws", name="mws")
                    nc.vector.tensor_copy(out=mws[:], in_=mw[:, 0:2])
                    xn = wsp.tile([P, P], F32, tag="xn", name="xn")
                    nc.scalar.activation(out=xn[:], in_=h_a[:, w, :],
                                         func=AF.Identity, scale=mws[:, 1:2],
                                         bias=mws[:, 0:1])
                    nc.vector.tensor_tensor(out=xn[:], in0=xn[:], in1=lnw,
                                            op=OP.mult)
                    nc.vector.tensor_tensor(out=xn[:], in0=xn[:], in1=lnb,
                                            op=OP.add)
                    # elu = max(x,0) + exp(min(x,0)) - 1
                    mn = wsp.tile([P, P], F32, tag="mn", name="mn")
                    nc.vector.tensor_scalar(out=mn[:], in0=xn[:], scalar1=0.0,
                                            scalar2=None, op0=OP.min)
                    nc.scalar.activation(out=mn[:], in_=mn[:], func=AF.Exp)
                    mx = wsp.tile([P, P], F32, tag="mx", name="mx")
                    nc.vector.tensor_scalar(out=mx[:], in0=xn[:], scalar1=0.0,
                                            scalar2=None, op0=OP.max)
                    nc.vector.tensor_tensor(out=mx[:], in0=mx[:], in1=mn[:],
                                            op=OP.add)
                    if add_resid:
                        nc.vector.tensor_scalar(out=mx[:], in0=mx[:],
                                                scalar1=1.0, scalar2=None,
                                                op0=OP.subtract)
                        rt = wsp.tile([P, P], BF, tag="rt", name="rt")
                        nc.sync.dma_start(out=rt[:],
                                          in_=rs_d[:, w * P:(w + 1) * P])
                        nc.vector.tensor_tensor(out=h_a[:, w, :], in0=mx[:],
                                                in1=rt[:], op=OP.add)
                    else:
                        nc.vector.tensor_scalar(out=h_a[:, w, :], in0=mx[:],
                                                scalar1=1.0, scalar2=None,
                                                op0=OP.subtract)
                    stage_n(w, ps)
            return st_n.get("xr_all")

        # ------------------------------------------------------------------
        def final_layer():
            wlf = wpk_s[:, 6 * P:6 * P + DF]
            wrf = wpk_s[:, 6 * P + DF:6 * P + 2 * DF]

            with tc.tile_pool(name="psf", bufs=1, space="PSUM") as ps:
                for g in groups:
                    gq, boff = grp_gathers(g)
                    for w in g["ws"]:
                        base = int(woff[w])
                        Tww = int(meta["Tw"][w])
                        xrfp = ps.tile([P, DF], F32, tag="ep", bufs=3,
                                       name="xrfp")
                        nc.tensor.matmul(out=xrfp[:], lhsT=hT[:, w, :],
                                         rhs=wrf, start=True, stop=True)
                        xrf = wsp.tile([P, DF], BF, tag="xrf", name="xrf")
                        nc.scalar.activation(out=xrf[:], in_=xrfp[:],
                                             func=AF.Identity)
                        ST = build_st(w, base, Tww)
                        ews2 = wsp.tile([2, TWMAX * P], BF, tag="ews2",
                                        name="ewsf")
                        nc.sync.dma_start(
                            out=ews2[0:2, :Tww * P],
                            in_=ew2_d[0:2, base * P:(base + Tww) * P])
                        cht = ps.tile([P, HEADS, P], F32, tag="cht",
                                      name="cht")
                        dnm = ps.tile([P, HEADS], F32, tag="fdnm", name="fdnm")
                        pend = []

                        def emit_cht(t0p, J, col0, lt0p, Bp, wqp, first,
                                     last, STx):
                            for j in range(J):
                                nc.tensor.matmul(
                                    out=cht[:].rearrange("p h c -> p (h c)"),
                                    lhsT=gq[:, col0 + j, :],
                                    rhs=Bp[:, j, :, :].rearrange(
                                        "p h c -> p (h c)"),
                                    start=(first and j == 0),
                                    stop=(last and j == J - 1))
                                nc.tensor.matmul(out=dnm[:],
                                                 lhsT=STx[:, :, lt0p + j],
                                                 rhs=wqp[:, j, :],
                                                 start=(first and j == 0),
                                                 stop=(last and j == J - 1))

                        lastsplit = max(si for si in range(S)
                                        if TS[si, w] > 0)
                        first = True
                        for si in range(S):
                            Th = int(TS[si, w])
                            if Th == 0:
                                continue
                            gq0 = g["smap"][si][w] + int(boff[si])
                            lt0 = int(run_off[si, w])
                            for t0p in range(0, Th, 2):
                                J = min(2, Th - t0p)
                                ea2 = wsp.tile([P, 2, DF], BF, tag="fea",
                                               bufs=3, name="fea")
                                for j in range(J):
                                    lt = lt0 + t0p + j
                                    col = gq0 + t0p + j
                                    gp = ps.tile([P, 2, P], BF, tag="pt",
                                                 bufs=2, name="gp")
                                    nc.tensor.transpose(out=gp[:, 0, :],
                                                        in_=gq[:, col, :],
                                                        identity=identb[:])
                                    nc.tensor.transpose(out=gp[:, 1, :],
                                                        in_=ST[:, :, lt],
                                                        identity=identb[:])
                                    gS = wsp.tile([P, 2, P], BF, tag="ghT",
                                                  bufs=3, name="ghT")
                                    nc.scalar.activation(out=gS[:],
                                                         in_=gp[:],
                                                         func=AF.Identity)
                                    ep = ps.tile([P, DF], F32, tag="ep",
                                                 bufs=3, name="fep")
                                    nc.tensor.matmul(
                                        out=ep[:],
                                        lhsT=ews2[0:2, lt * P:(lt + 1) * P],
                                        rhs=fwebb_s[0:2, :],
                                        start=True, stop=False)
                                    nc.tensor.matmul(out=ep[:],
                                                     lhsT=gS[:, 0, :],
                                                     rhs=wlf, start=False,
                                                     stop=False)
                                    nc.tensor.matmul(out=ep[:],
                                                     lhsT=gS[:, 1, :],
                                                     rhs=xrf[:], start=False,
                                                     stop=True)
                                    leaky(ea2[:, j, :], ep[:], [P, DF])
                                lg2 = wsp.tile([P, 2, DF], BF, tag="flg",
                                               bufs=3, name="flg")
                                nc.vector.tensor_tensor(
                                    out=lg2[:, :J, :], in0=ea2[:, :J, :],
                                    in1=attf_s[:, None, :].to_broadcast(
                                        [P, J, DF]),
                                    op=OP.mult)
                                lgr2 = wsp.tile([P, 2 * HEADS], BF,
                                                tag="flgr", name="flgr")
                                with nc.allow_low_precision(
                                        reason="bf16 head-logit reduce"):
                                    nc.vector.tensor_reduce(
                                        out=lgr2[:, :J * HEADS].rearrange(
                                            "p (j h) -> p j h", h=HEADS),
                                        in_=lg2[:, :J, :].rearrange(
                                            "p j (h c) -> p j h c", h=HEADS,
                                            c=P),
                                        axis=AX.X, op=OP.add)
                                wqp = wsp.tile([P, 2, HEADS], BF, tag="fwq",
                                               bufs=3, name="fwq")
                                nc.scalar.activation(
                                    out=wqp[:, :J, :],
                                    in_=lgr2[:, :J * HEADS].rearrange(
                                        "p (j h) -> p j h", h=HEADS),
                                    func=AF.Exp)
                                Bp = wsp.tile([P, 2, HEADS, P], BF, tag="fB",
                                              bufs=3, name="fB")
                                nc.vector.tensor_tensor(
                                    out=Bp[:, :J, :, :],
                                    in0=ST[:, :, lt0 + t0p:lt0 + t0p + J]
                                        .rearrange("p d j -> p j d")
                                        [:, :, None, :]
                                        .to_broadcast([P, J, HEADS, P]),
                                    in1=wqp[:, :J, :, None].to_broadcast(
                                        [P, J, HEADS, P]),
                                    op=OP.mult)
                                last = (si == lastsplit and
                                        t0p + J == Th)
                                if pend:
                                    emit_cht(*pend.pop())
                                pend.append((t0p, J, gq0 + t0p, lt0 + t0p,
                                             Bp, wqp, first, last, ST))
                                first = False
                        for args in pend:
                            emit_cht(*args)

                        # flush: nmr_h = ChT_h^T @ Wlf_h; out = bias +
                        #        mean_h numer/denom
                        chsb = wsp.tile([P, HEADS, P], BF, tag="chsb",
                                        name="chsb")
                        nc.scalar.activation(out=chsb[:], in_=cht[:],
                                             func=AF.Identity)
                        nmr = ps.tile([P, DF], F32, tag="ep", bufs=3,
                                      name="fnmr")
                        for h in range(HEADS):
                            nc.tensor.matmul(
                                out=nmr[:, h * P:(h + 1) * P],
                                lhsT=chsb[:, h, :],
                                rhs=wlf[:, h * P:(h + 1) * P],
                                start=True, stop=True)
                        rd = wsp.tile([P, HEADS], F32, tag="rd", name="frd")
                        nc.vector.tensor_scalar(out=rd[:], in0=dnm[:],
                                                scalar1=1e-16, scalar2=None,
                                                op0=OP.add)
                        nc.vector.reciprocal(out=rd[:], in_=rd[:])
                        nc.vector.tensor_scalar(out=rd[:], in0=rd[:],
                                                scalar1=1.0 / HEADS,
                                                scalar2=None, op0=OP.mult)
                        sc = wsp.tile([P, HEADS, P], F32, tag="sc", bufs=1,
                                      name="sc")
                        nc.vector.tensor_tensor(
                            out=sc[:],
                            in0=nmr[:].rearrange("p (h c) -> p h c", h=HEADS,
                                                 c=P),
                            in1=rd[:, :, None].to_broadcast([P, HEADS, P]),
                            op=OP.mult)
                        acc = wsp.tile([P, P], F32, tag="acc", name="acc")
                        nc.vector.tensor_reduce(
                            out=acc[:], in_=sc[:].rearrange("p h c -> p c h"),
                            axis=AX.X, op=OP.add)
                        nc.vector.tensor_tensor(out=acc[:], in0=acc[:],
                                                in1=fpk_s[:], op=OP.add)
                        nc.sync.dma_start(out=out_d[w * P:(w + 1) * P, :],
                                          in_=acc[:])

        # ---- the 4 layers (layer li+1's staging fused into li's P3) ----
        with tc.tile_pool(name="psS", bufs=1, space="PSUM") as psS:
            st0, stage0 = make_stager(0)
            for w in range(NW):
                stage0(w, psS)
        xr = st0["xr_all"]
        xr = hidden_layer(0, False, xr, 1)
        xr = hidden_layer(1, True, xr, 2)
        hidden_layer(2, False, xr, 3)
        final_layer()

    nc.compile()
    return nc


# ----------------------------------------------------------------------------
# Host-side driver
# ----------------------------------------------------------------------------

def _repP(v):
    v = np.asarray(v, np.float32).reshape(-1)
    return np.broadcast_to(v, (P, v.shape[0]))


def _winmaj(arr, lo_c, hi_c, NW):
    """[n, P] node-major slice -> [P, NW*P] window-major (padded)."""
    out = np.zeros((NW * P, P), np.float32)
    out[:hi_c - lo_c] = arr[lo_c:hi_c]
    return np.ascontiguousarray(
        out.reshape(NW, P, P).transpose(1, 0, 2).reshape(P, NW * P))


def make_in_maps(meta, inputs):
    NW, TT = meta["NW"], meta["TT"]
    lo, hi = meta["lo"], meta["hi"]
    x = np.asarray(inputs["x"], np.float32)
    resid = np.asarray(inputs["residual"], np.float32)
    ew = np.asarray(inputs["edge_weight"], np.float32)

    att = np.asarray(inputs["att"], np.float32)      # (3, H, C)
    attf = np.asarray(inputs["att_f"], np.float32)   # (H, DOUT)
    bl = np.asarray(inputs["bl"], np.float32)
    br = np.asarray(inputs["br"], np.float32)
    bias = np.asarray(inputs["bias"], np.float32)
    blf = np.asarray(inputs["bl_f"], np.float32)
    brf = np.asarray(inputs["br_f"], np.float32)
    biasf = np.asarray(inputs["bias_f"], np.float32)

    bf16 = ml_dtypes.bfloat16
    # hidden features stored (c h)-interleaved so the DVE alpha-weighting
    # multiply has a packed last dim (2x mode); PRM[c*H+h] = h*CH+c
    PRM = np.array([h * CH + c for c in range(CH) for h in range(HEADS)])
    wpk = np.concatenate(
        [np.asarray(inputs["Wl"], np.float32)[i][:, PRM] for i in range(3)]
        + [np.asarray(inputs["Wr"], np.float32)[i][:, PRM] for i in range(3)]
        + [np.asarray(inputs["Wl_f"], np.float32),
           np.asarray(inputs["Wr_f"], np.float32)], axis=1).astype(bf16)
    cpk = np.concatenate(
        [_repP(inputs["ln_w"][i]) for i in range(3)]
        + [_repP(inputs["ln_b"][i]) for i in range(3)]
        + [_repP(bias[i] + bl[i]) for i in range(3)], axis=1).astype(np.float32)
    apk = np.concatenate([_repP(att[i].reshape(-1)[PRM]) for i in range(3)],
                         axis=1).astype(bf16)
    We = np.asarray(inputs["We"], np.float32)
    webb = np.zeros((5, 12 * P), np.float32)
    for l in range(3):
        webb[0, l * 4 * P:(l + 1) * 4 * P] = np.tile((bl[l] + br[l])[PRM], 4)
        for r in range(4):
            webb[1 + r, l * 4 * P + r * P:l * 4 * P + (r + 1) * P] = \
                We[l].reshape(P)[PRM]
    fwebb = np.stack([blf + brf,
                      np.asarray(inputs["We_f"], np.float32).reshape(DF)])
    attf_rep = _repP(attf).astype(bf16)
    biaf_eff = biasf + blf.reshape(HEADS, -1).mean(axis=0)
    fpk = _repP(biaf_eff).astype(np.float32)

    common = dict(invd=meta["invd"].astype(np.float32), cpk=cpk, apk=apk,
                  wpk=wpk, attf=attf_rep, fpk=fpk,
                  webb=webb.astype(bf16), fwebb=fwebb.astype(bf16))

    S = meta["S"]
    woff, TS, run_off = meta["woff"], meta["TS"], meta["run_off"]
    nqcum, qoff, NQTOT = meta["nqcum"], meta["qoff"], meta["NQTOT"]

    in_maps = []
    for c in range(NCORES):
        ewc = np.zeros(TT * P, np.float32)
        m = meta["ewsl"][c] >= 0
        ewc[m] = ew[meta["ewsl"][c][m]]
        ew2 = np.stack([np.ones(TT * P, np.float32), ewc])
        ew5 = np.zeros((5, NQTOT * P), np.float32)
        ew5[0] = 1.0
        for w in range(NW):
            for si in range(S):
                Th = int(TS[si, w])
                lt0 = int(run_off[si, w])
                qq0 = int(qoff[w]) + int(nqcum[si, w])
                for q0 in range(0, Th, 4):
                    qq = qq0 + q0 // 4
                    for r in range(min(4, Th - q0)):
                        gt = int(woff[w]) + lt0 + q0 + r
                        ew5[1 + r, qq * P:(qq + 1) * P] = \
                            ewc[gt * P:(gt + 1) * P]
        in_maps.append(dict(
            h0s=_winmaj(x, lo[c], hi[c], NW).astype(bf16),
            rs=_winmaj(resid, lo[c], hi[c], NW).astype(bf16),
            dsti=meta["dsti"][c].astype(bf16),
            idx=np.tile(meta["idxm"][c], (8, 1)),
            ew5=ew5.astype(bf16),
            ew2=ew2.astype(bf16),
            gmat=meta["gmat"][c].astype(bf16),
            gmatT=meta["gmatT"][c].astype(bf16),
            **common))
    return in_maps


def assemble(meta, results):
    N = meta["N"]
    lo, hi = meta["lo"], meta["hi"]
    out = np.zeros((N, P), np.float32)
    for c in range(NCORES):
        n = int(hi[c] - lo[c])
        out[lo[c]:hi[c]] = results[c]["out"][:n]
    return out


_CACHE = {}


def kernel(**inputs):
    ei = np.asarray(inputs["edge_index"])
    bt = np.asarray(inputs["batch"])
    key = (ei.shape, bt.shape, hash(ei.tobytes()), hash(bt.tobytes()))
    if key not in _CACHE:
        meta = build_meta(ei, bt)
        nc = build_program(meta)
        _CACHE[key] = (meta, nc)
    meta, nc = _CACHE[key]
    in_maps = make_in_maps(meta, inputs)
    res = run_bass_kernel_spmd(nc, in_maps, list(range(NCORES)))
    return assemble(meta, res.results)



# revision 29
# speedup vs baseline: 1.0003x; 1.0003x over previous
"""GATv2 backbone (4 layers) on 8 Trainium2 NeuronCores.

Strategy (v4):
  * v3 + cross-layer phase fusion: each layer's normalize (P3) stages the
    NEXT layer's tables per window (transpose + xl/xr projection + DRAM
    bounce) and fires the per-band AllGathers early, removing the serial
    P3->P0 boundary. Gathers issue with single_packet=False (descriptors
    spread over all 16 SDMA engines); deeper tile buffering (gq bufs=3,
    per-quad chain bufs=4) keeps more quads in flight across the
    10-step cross-engine dependency chain.

Strategy (v3):
  * Nodes partitioned into 8 contiguous ranges (edge-balanced); edges owned
    by the dst core, grouped by 128-node dst windows. Windows are divided
    into SPLITS contiguous bands; each band's node table (<=32767 rows, so
    int16 gather indices work) is AllGathered separately, and the AllGathers
    are issued as soon as their band's xl shard is staged -- they pipeline
    with P0 and with edge-phase compute of earlier bands.
  * Gathers: nc.gpsimd.dma_gather, <=8 tiles (1024 rows) per op, one op per
    (window-group, band, chunk) -- ~130 ops/layer vs 850 indirect DMAs.
  * Hidden layers gather xl rows (256B); the final layer gathers h rows
    (256B) and computes xlf = h @ Wlf on-chip (4x less gather + AllGather
    traffic), with the numerator factored as (B_h^T @ gh) @ Wlf_h.
  * One-hot ST built bf16 in [e, d, t] layout (packed last dim -> DVE 2x
    mode); S = PE transpose of ST slices, PSUM->SBUF copies on the ACT
    engine. Edge-weight rank-1s and both linear biases fold into a single
    [ones; ew-rows] @ [bias; We-blocks] matmul per quad; numerator bias
    recovered via sum(alpha)=1. nmr|dnm share one PSUM accumulation matmul.
  * leaky_relu via Prelu (same ACT table set as Exp/Square -> no reloads).
"""

import contextlib
import os

import ml_dtypes
import numpy as np

from concourse import bass, bacc, mybir, tile
from concourse.bass_utils import run_bass_kernel_spmd
from concourse.masks import make_identity

P = 128
NCORES = 8
GMAX = 50
HEADS = 4
DHID = 128
CH = DHID // HEADS          # 32
DF = 512
NEG = 0.2
EPS = 1e-5
GW = 1                      # windows per gather group
MAXT = 8                    # tiles per dma_gather op (1024 idxs, HW limit)
NSPLIT = 4                  # table bands (pipelined AllGathers)
NQUEUES = 4                 # SWDGE queues for gather DMAs
USE_LRELU = os.environ.get("K_NO_PRELU", "") != "1"

F32 = mybir.dt.float32
BF = mybir.dt.bfloat16
I32 = mybir.dt.int32
I16 = mybir.dt.int16
AX = mybir.AxisListType
OP = mybir.AluOpType
AF = mybir.ActivationFunctionType


# ----------------------------------------------------------------------------
# Host preprocessing: graph partitioning + static schedule
# ----------------------------------------------------------------------------

def build_meta(edge_index, batch):
    N = batch.shape[0]
    E = edge_index.shape[1]
    src = np.asarray(edge_index[0], dtype=np.int64)
    dst = np.asarray(edge_index[1], dtype=np.int64)
    batch = np.asarray(batch, dtype=np.int64)

    deg = np.bincount(dst, minlength=N)
    cum = np.concatenate([[0], np.cumsum(deg)])

    bounds = [0]
    for c in range(1, NCORES):
        n = int(np.searchsorted(cum, c * E / NCORES))
        bounds.append(min(max(n, bounds[-1] + 1), N - (NCORES - c)))
    bounds.append(N)
    lo = np.array(bounds[:-1])
    hi = np.array(bounds[1:])

    NW = int(max((hi - lo + P - 1) // P))
    NPAD = NW * P
    S = min(NSPLIT, NW)
    swin = [a for a in np.array_split(np.arange(NW), S)]
    sbase = np.array([int(a[0]) for a in swin])
    scount = np.array([len(a) for a in swin])
    TBLROWS = NCORES * scount * P
    assert (TBLROWS <= 32767).all(), TBLROWS
    split_id = np.zeros(NW, np.int64)
    for si, a in enumerate(swin):
        split_id[a] = si

    core_of = np.repeat(np.arange(NCORES), (hi - lo))
    off = np.arange(N) - lo[core_of]
    wof = off // P                       # window-within-core (may pad-overflow
    wof = np.minimum(wof, NW - 1)        # never: off < NPAD)
    nsp = split_id[wof]
    nrow = core_of * scount[nsp] * P + (off - sbase[nsp] * P)

    order = np.argsort(dst, kind="stable")

    cnt = np.zeros((S, NCORES, NW), np.int64)
    lists = {}
    for c in range(NCORES):
        for w in range(NW):
            a = lo[c] + w * P
            b = min(a + P, hi[c])
            if a >= b:
                lists[(c, w)] = [np.empty(0, np.int64)] * S
                continue
            ids = order[int(cum[a]):int(cum[b])]
            sp = nsp[src[ids]]
            per = []
            for si in range(S):
                idsS = ids[sp == si]
                idsS = idsS[np.argsort(nrow[src[idsS]], kind="stable")]
                per.append(idsS)
                cnt[si, c, w] = len(idsS)
            lists[(c, w)] = per

    TS = ((cnt.max(axis=1) + P - 1) // P).astype(np.int64)    # [S, NW]
    for w in range(NW):
        if TS[:, w].sum() == 0:
            TS[0, w] = 1
    run_off = np.zeros((S, NW), np.int64)
    for si in range(1, S):
        run_off[si] = run_off[si - 1] + TS[si - 1]
    Tw = TS.sum(axis=0)
    woff = np.concatenate([[0], np.cumsum(Tw)]).astype(np.int64)
    TT = int(woff[-1])

    dsti = np.full((NCORES, P, TT), -1, np.int64)
    ewsl = np.full((NCORES, TT * P), -1, np.int64)
    srow = np.zeros((NCORES, TT * P), np.int64)
    for c in range(NCORES):
        for w in range(NW):
            for si in range(S):
                ids_h = lists[(c, w)][si]
                n = len(ids_h)
                if n == 0:
                    continue
                tb = int(woff[w] + run_off[si, w])
                slot = np.arange(n)
                tt = tb + slot // P
                pp = slot % P
                dsti[c, pp, tt] = dst[ids_h] - (lo[c] + w * P)
                flat = tt * P + pp
                ewsl[c, flat] = ids_h
                srow[c, flat] = nrow[src[ids_h]]

    # per-window quad schedule (band-major)
    nqS = ((TS + 3) // 4).astype(np.int64)            # [S, NW]
    nqcum = np.zeros((S, NW), np.int64)
    for si in range(1, S):
        nqcum[si] = nqcum[si - 1] + nqS[si - 1]
    nquad = nqS.sum(axis=0)
    NQBMAX = int(nqS.max())
    qoff = np.concatenate([[0], np.cumsum(nquad)]).astype(np.int64)
    NQTOT = int(qoff[-1])
    NQMAX = int(nquad.max())

    # gather-op schedule (static, identical across cores)
    groups = []
    icols = 0
    for g0 in range(0, NW, GW):
        ws = list(range(g0, min(g0 + GW, NW)))
        smap = [dict() for _ in range(S)]
        tgb = []
        ops = []
        for si in range(S):
            run = 0
            for w in ws:
                smap[si][w] = run
                run += int(TS[si, w])
            tgb.append(run)
            tiles = []
            for w in ws:
                for t in range(int(TS[si, w])):
                    gt = int(woff[w] + run_off[si, w]) + t
                    tiles.append((gt, smap[si][w] + t))
            for i in range(0, len(tiles), MAXT):
                ch = tiles[i:i + MAXT]
                ops.append(dict(split=si, coff=icols, nt=len(ch),
                                gq0=ch[0][1], gtiles=[x[0] for x in ch]))
                icols += len(ch) * 8
        groups.append(dict(ws=ws, tgb=tgb, smap=smap, ops=ops))
    ICOLS = icols
    TGMAX = max(max(g["tgb"]) for g in groups)
    TGFULL = max(sum(g["tgb"]) for g in groups)
    TWMAX = int(Tw.max())
    fnz = np.array([min(si for si in range(S) if TS[si, w] > 0)
                    for w in range(NW)])
    lnz = np.array([max(si for si in range(S) if TS[si, w] > 0)
                    for w in range(NW)])

    idxm = np.zeros((NCORES, 16, ICOLS), np.int16)
    for c in range(NCORES):
        for g in groups:
            for op in g["ops"]:
                nt = op["nt"]
                vals = np.zeros(nt * P, np.int64)
                for j, gt in enumerate(op["gtiles"]):
                    vals[j * P:(j + 1) * P] = srow[c, gt * P:(gt + 1) * P]
                idxm[c, :, op["coff"]:op["coff"] + nt * 8] = (
                    vals.reshape(nt * 8, 16).T.astype(np.int16))

    gmat = np.zeros((NCORES, P, NW * GMAX), np.float32)
    gmatT = np.zeros((NCORES, GMAX, NW * P), np.float32)
    for c in range(NCORES):
        nreal = int(hi[c] - lo[c])
        g = batch[lo[c]:hi[c]]
        r = np.arange(nreal)
        gmat[c, r % P, (r // P) * GMAX + g] = 1.0
        gmatT[c, g, (r // P) * P + (r % P)] = 1.0

    cntg = np.bincount(batch, minlength=GMAX).astype(np.float32)
    invd = (1.0 / (np.maximum(cntg, 1.0) * DHID)).reshape(1, GMAX)

    return dict(N=N, E=E, NW=NW, NPAD=NPAD, S=S, swin=swin, sbase=sbase,
                scount=scount, TBLROWS=TBLROWS, TT=TT, ICOLS=ICOLS,
                TGMAX=TGMAX, TWMAX=TWMAX, TS=TS, run_off=run_off, Tw=Tw,
                woff=woff, lo=lo, hi=hi, nqS=nqS, nqcum=nqcum, nquad=nquad,
                qoff=qoff, NQTOT=NQTOT, NQMAX=NQMAX, NQBMAX=NQBMAX,
                TGFULL=TGFULL,
                groups=groups, fnz=fnz, lnz=lnz,
                dsti=dsti, ewsl=ewsl, idxm=idxm, gmat=gmat, gmatT=gmatT,
                invd=invd)


# ----------------------------------------------------------------------------
# Bass program
# ----------------------------------------------------------------------------

def build_program(meta):
    NW, NPAD, TT = meta["NW"], meta["NPAD"], meta["TT"]
    S, sbase, scount = meta["S"], meta["sbase"], meta["scount"]
    TBLROWS, ICOLS = meta["TBLROWS"], meta["ICOLS"]
    TGMAX, TWMAX = meta["TGMAX"], meta["TWMAX"]
    TS, run_off, woff = meta["TS"], meta["run_off"], meta["woff"]
    nqS, nqcum, nquad, qoff = (meta["nqS"], meta["nqcum"], meta["nquad"],
                               meta["qoff"])
    NQTOT, NQMAX = meta["NQTOT"], meta["NQMAX"]
    NQBMAX = meta["NQBMAX"]
    TGFULL = meta["TGFULL"]
    groups = meta["groups"]
    swin = meta["swin"]
    fnz, lnz = meta["fnz"], meta["lnz"]

    nc = bacc.Bacc("TRN2", target_bir_lowering=False, debug=False,
                   enable_asserts=False, num_devices=NCORES,
                   num_swdge_queues=1)

    h0s_d = nc.dram_tensor("h0s", [P, NW * P], BF, kind="ExternalInput")
    rs_d = nc.dram_tensor("rs", [P, NW * P], BF, kind="ExternalInput")
    dsti_d = nc.dram_tensor("dsti", [P, TT], BF, kind="ExternalInput")
    idx_d = nc.dram_tensor("idx", [P, ICOLS], I16, kind="ExternalInput")
    ew5_d = nc.dram_tensor("ew5", [5, NQTOT * P], BF, kind="ExternalInput")
    ew2_d = nc.dram_tensor("ew2", [2, TT * P], BF, kind="ExternalInput")
    gmat_d = nc.dram_tensor("gmat", [P, NW * GMAX], BF, kind="ExternalInput")
    gmatT_d = nc.dram_tensor("gmatT", [GMAX, NW * P], BF, kind="ExternalInput")
    invd_d = nc.dram_tensor("invd", [1, GMAX], F32, kind="ExternalInput")
    cpk_d = nc.dram_tensor("cpk", [P, 9 * P], F32, kind="ExternalInput")
    apk_d = nc.dram_tensor("apk", [P, 3 * P], BF, kind="ExternalInput")
    wpk_d = nc.dram_tensor("wpk", [P, 6 * P + 2 * DF], BF, kind="ExternalInput")
    attf_d = nc.dram_tensor("attf", [P, DF], BF, kind="ExternalInput")
    fpk_d = nc.dram_tensor("fpk", [P, P], F32, kind="ExternalInput")
    webb_d = nc.dram_tensor("webb", [5, 12 * P], BF, kind="ExternalInput")
    fwebb_d = nc.dram_tensor("fwebb", [2, DF], BF, kind="ExternalInput")
    out_d = nc.dram_tensor("out", [NPAD, P], F32, kind="ExternalOutput")

    with tile.TileContext(nc) as tc, contextlib.ExitStack() as ctx:
        dram = ctx.enter_context(tc.tile_pool(name="dram", bufs=1, space="DRAM"))
        cst = ctx.enter_context(tc.tile_pool(name="cst", bufs=1))
        per = ctx.enter_context(tc.tile_pool(name="per", bufs=1))
        wsp = ctx.enter_context(tc.tile_pool(name="wsp", bufs=2))
        gpo = ctx.enter_context(tc.tile_pool(name="gpo", bufs=2))

        xl_b = dram.tile([NPAD, P], BF)
        tbl = [nc.dram_tensor(f"tbl{si}", [int(TBLROWS[si]), P], BF,
                              kind="Internal", addr_space="Shared")
               for si in range(S)]
        st_b = dram.tile([2, GMAX], F32)
        st_o = dram.tile([2, GMAX], F32)
        cgroups = [list(range(NCORES))]

        # --- constants / residents ---
        ident = cst.tile([P, P], F32)
        make_identity(nc, ident[:])
        identb = cst.tile([P, P], BF)
        nc.vector.tensor_copy(out=identb[:], in_=ident[:])
        iota_row = cst.tile([P, P], I32)
        nc.gpsimd.iota(iota_row[:], pattern=[[1, P]], base=0,
                       channel_multiplier=0)
        iota_rowb = cst.tile([P, P], BF)
        nc.vector.tensor_copy(out=iota_rowb[:], in_=iota_row[:])
        iota3 = cst.tile([P, P, TWMAX], BF)
        nc.vector.tensor_copy(
            out=iota3[:],
            in_=iota_rowb[:, :, None].to_broadcast([P, P, TWMAX]))
        epsc = cst.tile([P, 1], F32)
        nc.vector.memset(epsc[:], EPS)
        invd = cst.tile([1, GMAX], F32)
        nc.sync.dma_start(out=invd[:], in_=invd_d[:, :])
        wpk_s = cst.tile([P, 6 * P + 2 * DF], BF)
        nc.sync.dma_start(out=wpk_s[:], in_=wpk_d[:, :])
        dsti_s = cst.tile([P, TT], BF)
        nc.sync.dma_start(out=dsti_s[:], in_=dsti_d[:, :])
        idx_s = cst.tile([P, ICOLS], I16)
        nc.sync.dma_start(out=idx_s[:], in_=idx_d[:, :])
        gmat_s = cst.tile([P, NW * GMAX], BF)
        nc.sync.dma_start(out=gmat_s[:], in_=gmat_d[:, :])
        gmatT_s = cst.tile([GMAX, NW * P], BF)
        nc.sync.dma_start(out=gmatT_s[:], in_=gmatT_d[:, :])
        cpk_s = cst.tile([P, 9 * P], F32)
        nc.sync.dma_start(out=cpk_s[:], in_=cpk_d[:, :])
        apk_s = cst.tile([P, 3 * P], BF)
        nc.sync.dma_start(out=apk_s[:], in_=apk_d[:, :])
        attf_s = cst.tile([P, DF], BF)
        nc.sync.dma_start(out=attf_s[:], in_=attf_d[:, :])
        fpk_s = cst.tile([P, P], F32)
        nc.sync.dma_start(out=fpk_s[:], in_=fpk_d[:, :])
        webb_s = cst.tile([5, 12 * P], BF)
        nc.sync.dma_start(out=webb_s[:], in_=webb_d[:, :])
        fwebb_s = cst.tile([2, DF], BF)
        nc.sync.dma_start(out=fwebb_s[:], in_=fwebb_d[:, :])

        h_a = per.tile([P, NW, P], BF, tag="h_a")
        hT = per.tile([P, NW, P], BF, tag="hT")
        nc.sync.dma_start(out=h_a[:, :, :],
                          in_=h0s_d[:, :].rearrange("p (w f) -> p w f", w=NW))

        def leaky(dst_ap, src_ap, shape):
            if USE_LRELU:
                # Prelu == leaky relu with param alpha; unlike Lrelu it is in
                # the same ACT table set as Exp/Square -> no table reloads.
                nc.scalar.activation(out=dst_ap, in_=src_ap, func=AF.Prelu,
                                     alpha=NEG)
            else:
                r = wsp.tile(shape, F32, tag="lrtmp", bufs=1, name="lr")
                rr = r[tuple(slice(0, s) for s in dst_ap.shape)]
                nc.scalar.activation(out=rr, in_=src_ap, func=AF.Relu,
                                     scale=-(1.0 - NEG))
                nc.vector.tensor_tensor(out=dst_ap, in0=src_ap, in1=rr,
                                        op=OP.add)

        split_last = {int(a[-1]): si for si, a in enumerate(swin)}
        SGW = 2                 # staging chunk (windows per xl_b DMA)
        breaks = set(range(SGW - 1, NW, SGW)) | set(split_last) | {NW - 1}

        def emit_ag(si):
            a = int(sbase[si]) * P
            b = a + int(scount[si]) * P
            nc.gpsimd.collective_compute(
                "AllGather", OP.bypass, replica_groups=cgroups,
                ins=[xl_b[a:b, :].opt()], outs=[tbl[si][:, :].opt()])

        qctr = [0]

        def grp_gathers(g, band=None):
            if band is not None:
                gq = gpo.tile([P, TGMAX, P], BF, tag="gq", name="gq",
                              bufs=5)
            else:
                gq = gpo.tile([P, TGFULL, P], BF, tag="gqf", name="gqf",
                              bufs=3)
            boff = np.concatenate([[0], np.cumsum(g["tgb"])]).astype(int)
            for op in g["ops"]:
                if band is not None and op["split"] != band:
                    continue
                g0 = op["gq0"] + (0 if band is not None
                                  else int(boff[op["split"]]))
                nc.gpsimd.dma_gather(
                    gq[:, g0:g0 + op["nt"], :],
                    tbl[op["split"]][:, :],
                    idx_s[:, op["coff"]:op["coff"] + op["nt"] * 8],
                    op["nt"] * P, op["nt"] * P, P,
                    single_packet=False)
            return gq, boff

        def build_st(w, t0, Tn):
            """ST2[e, d, t] one-hot (bf16, packed last dim -> DVE 2x)."""
            ST = wsp.tile([P, P, TWMAX], BF, tag="ST", name="ST")
            nc.vector.tensor_tensor(
                out=ST[:, :, :Tn],
                in0=iota3[:, :, :Tn],
                in1=dsti_s[:, None, t0:t0 + Tn]
                    .to_broadcast([P, P, Tn]),
                op=OP.is_equal)
            return ST

        # ------------------------------------------------------------------
        def make_stager(li):
            """Staging for layer li's tables: transpose h, project (hidden)
            or copy (final), bounce to DRAM, fire per-band AllGathers.
            Called per window, fused into the previous layer's P3."""
            st = dict(run=False, w0=0, xsg=None)
            if li < 3:
                st["xr_all"] = wsp.tile([P, NW, P], BF, tag="xra", bufs=1,
                                        name="xra")
                wl = wpk_s[:, li * P:(li + 1) * P]
                wr = wpk_s[:, (3 + li) * P:(4 + li) * P]

            def stage(w, ps):
                nc.sync.dma_start(out=hT[:, w, :], in_=h_a[:, w, :],
                                  transpose=True)
                if not st["run"]:
                    st["run"] = True
                    st["w0"] = w
                    if li < 3:
                        st["xsg"] = wsp.tile([P, SGW, P], BF, tag="xsg",
                                             name="xsg")
                w0 = st["w0"]
                if li < 3:
                    xp = ps.tile([P, P], F32, tag="px", bufs=1, name="px")
                    nc.tensor.matmul(out=xp[:], lhsT=hT[:, w, :], rhs=wl,
                                     start=True, stop=True)
                    nc.scalar.activation(out=st["xsg"][:, w - w0, :],
                                         in_=xp[:], func=AF.Identity)
                    xrp = ps.tile([P, P], F32, tag="px", bufs=1, name="xrp")
                    nc.tensor.matmul(out=xrp[:], lhsT=hT[:, w, :], rhs=wr,
                                     start=True, stop=True)
                    nc.scalar.activation(out=st["xr_all"][:, w, :],
                                         in_=xrp[:], func=AF.Identity)
                    if w in breaks:
                        nc.sync.dma_start(
                            out=xl_b[w0 * P:(w + 1) * P, :].rearrange(
                                "(w p) f -> p w f", p=P),
                            in_=st["xsg"][:, :w - w0 + 1, :])
                        st["run"] = False
                else:
                    if w in breaks:
                        nc.sync.dma_start(
                            out=xl_b[w0 * P:(w + 1) * P, :].rearrange(
                                "(w p) f -> p w f", p=P),
                            in_=h_a[:, w0:w + 1, :])
                        st["run"] = False
                if w in split_last:
                    emit_ag(split_last[w])
            return st, stage

        def hidden_layer(li, add_resid, xr_all, next_li):
            attr = apk_s[:, li * P:(li + 1) * P]
            lnw = cpk_s[:, li * P:(li + 1) * P]
            lnb = cpk_s[:, (3 + li) * P:(4 + li) * P]
            bia = cpk_s[:, (6 + li) * P:(7 + li) * P]

            with tc.tile_pool(name=f"ps{li}", bufs=1, space="PSUM") as ps:
                # P2: edge pipeline, band-major so AllGather si+1 overlaps
                # band-si compute; per-window numerators accumulate in SBUF.
                nd_all = wsp.tile([P, NW, P + HEADS], BF, tag="nda",
                                  bufs=1, name="nda")
                for band in range(S):
                    for g in groups:
                        if all(TS[band, w] == 0 for w in g["ws"]):
                            continue
                        gq, _ = grp_gathers(g, band)
                        gqv = gq[:].rearrange("p t (c h) -> p t c h",
                                              h=HEADS, c=CH)
                        for w in g["ws"]:
                            Th = int(TS[band, w])
                            if Th == 0:
                                continue
                            t0g = int(woff[w] + run_off[band, w])
                            ST = build_st(w, t0g, Th)
                            nqb = (Th + 3) // 4
                            qb0 = int(qoff[w] + nqcum[band, w])
                            ews5 = wsp.tile([5, NQBMAX * P], BF, tag="ews",
                                            bufs=4, name="ews")
                            nc.sync.dma_start(
                                out=ews5[0:5, :nqb * P],
                                in_=ew5_d[0:5, qb0 * P:(qb0 + nqb) * P])
                            nd = ps.tile([P, P + HEADS], F32, tag="nd",
                                         name="nd")
                            gq0 = g["smap"][band][w]

                            def emit_nd(q0, Q, mmw):
                                for t in range(Q):
                                    nc.tensor.matmul(
                                        out=nd[:], lhsT=ST[:, :, q0 + t],
                                        rhs=mmw[:, t, :],
                                        start=(q0 + t == 0),
                                        stop=(q0 + t == Th - 1))

                            pend = None
                            for q0 in range(0, Th, 4):
                                Q = min(4, Th - q0)
                                qq = q0 // 4
                                Ssb = wsp.tile([P, 4, P], BF, tag="ssb",
                                               bufs=5, name="ssb")
                                sp = ps.tile([P, 4, P], BF, tag="pt",
                                             bufs=3, name="sp")
                                for t in range(Q):
                                    nc.tensor.transpose(
                                        out=sp[:, t, :],
                                        in_=ST[:, :, q0 + t],
                                        identity=identb[:])
                                nc.scalar.activation(out=Ssb[:, :Q, :],
                                                     in_=sp[:, :Q, :],
                                                     func=AF.Identity)
                                ep = ps.tile([P, 4 * P], F32, tag="ep",
                                             bufs=2, name="ep")
                                # ef + biases first (host data, always ready)
                                # and the gather-dependent copy last, so the
                                # in-order PE queue head never parks on a
                                # not-yet-landed DMA.
                                nc.tensor.matmul(
                                    out=ep[:, :Q * P],
                                    lhsT=ews5[0:Q + 1, qq * P:(qq + 1) * P],
                                    rhs=webb_s[0:Q + 1,
                                               li * 4 * P:li * 4 * P + Q * P],
                                    start=True, stop=False)
                                for t in range(Q):
                                    blk = ep[:, t * P:(t + 1) * P]
                                    nc.tensor.matmul(out=blk,
                                                     lhsT=Ssb[:, t, :],
                                                     rhs=xr_all[:, w, :],
                                                     start=False, stop=False)
                                nc.tensor.matmul(
                                    out=ep[:, :Q * P], lhsT=identb[:],
                                    rhs=gq[:, gq0 + q0:gq0 + q0 + Q, :],
                                    start=False, stop=True)
                                ea = wsp.tile([P, 4 * P], BF, tag="ea", bufs=5,
                                              name="ea")
                                leaky(ea[:, :Q * P], ep[:, :Q * P], [P, 4 * P])
                                lg = wsp.tile([P, 4 * P], BF, tag="lg", bufs=5,
                                              name="lg")
                                nc.vector.tensor_tensor(
                                    out=lg[:, :Q * P], in0=ea[:, :Q * P],
                                    in1=attr[:, None, :].to_broadcast(
                                        [P, Q, P]),
                                    op=OP.mult)
                                lgr = wsp.tile([P, 4 * HEADS], F32, tag="lgr",
                                               bufs=4, name="lgr")
                                nc.vector.tensor_reduce(
                                    out=lgr[:, :Q * HEADS].rearrange(
                                        "p (t h) -> p t h", h=HEADS),
                                    in_=lg[:].rearrange(
                                        "p (t c h) -> p t h c", h=HEADS,
                                        c=CH)[:, :Q, :, :],
                                    axis=AX.X, op=OP.add)
                                mmw = wsp.tile([P, 4, P + HEADS], BF,
                                               tag="mm", bufs=4, name="mm")
                                nc.scalar.activation(
                                    out=mmw[:, :Q, P:P + HEADS],
                                    in_=lgr[:, :Q * HEADS].rearrange(
                                        "p (t h) -> p t h", h=HEADS),
                                    func=AF.Exp)
                                nc.vector.tensor_tensor(
                                    out=mmw[:, :Q, 0:P].rearrange(
                                        "p q (c h) -> p q c h", h=HEADS,
                                        c=CH),
                                    in0=gqv[:, gq0 + q0:gq0 + q0 + Q, :, :],
                                    in1=mmw[:, :Q, None, P:P + HEADS]
                                        .to_broadcast([P, Q, CH, HEADS]),
                                    op=OP.mult)
                                if pend is not None:
                                    emit_nd(*pend)
                                pend = (q0, Q, mmw)
                            if pend is not None:
                                emit_nd(*pend)
                            if band == int(fnz[w]):
                                nc.scalar.activation(out=nd_all[:, w, :],
                                                     in_=nd[:],
                                                     func=AF.Identity)
                            else:
                                nc.vector.tensor_tensor(
                                    out=nd_all[:, w, :], in0=nd_all[:, w, :],
                                    in1=nd[:], op=OP.add)

                # window flush + LN stats
                stp = ps.tile([2, GMAX], F32, tag="stats", name="stp")
                for w in range(NW):
                    rd = wsp.tile([P, HEADS], F32, tag="rd", name="rd")
                    nc.vector.tensor_scalar(out=rd[:],
                                            in0=nd_all[:, w, P:P + HEADS],
                                            scalar1=1e-16, scalar2=None,
                                            op0=OP.add)
                    nc.vector.reciprocal(out=rd[:], in_=rd[:])
                    oT = wsp.tile([P, HEADS, CH], F32, tag="oT", name="oT")
                    nc.vector.tensor_tensor(
                        out=oT[:],
                        in0=nd_all[:, w, :P].rearrange("p (c h) -> p h c",
                                                       h=HEADS, c=CH),
                        in1=rd[:, :, None].to_broadcast([P, HEADS, CH]),
                        op=OP.mult)
                    nc.vector.tensor_tensor(
                        out=h_a[:, w, :],
                        in0=oT[:].rearrange("p h c -> p (h c)"),
                        in1=bia, op=OP.add)
                    s12 = wsp.tile([P, 2], F32, tag="s12", name="s12")
                    nc.vector.tensor_reduce(out=s12[:, 0:1],
                                            in_=h_a[:, w, :],
                                            axis=AX.X, op=OP.add)
                    sqj = wsp.tile([P, P], BF, tag="sqj", name="sqj")
                    nc.scalar.activation(out=sqj[:], in_=h_a[:, w, :],
                                         func=AF.Square,
                                         accum_out=s12[:, 1:2])
                    s12b = wsp.tile([P, 2], BF, tag="s12b", name="s12b")
                    nc.vector.tensor_copy(out=s12b[:], in_=s12[:])
                    nc.tensor.matmul(
                        out=stp[:, :], lhsT=s12b[:],
                        rhs=gmat_s[:, w * GMAX:(w + 1) * GMAX],
                        start=(w == 0), stop=(w == NW - 1))

                # P3: stats -> mean/rstd -> normalize + elu
                sts = wsp.tile([2, GMAX], F32, tag="sts", name="sts")
                nc.vector.tensor_copy(out=sts[:], in_=stp[:])
                nc.sync.dma_start(out=st_b[:, :], in_=sts[:])
                nc.gpsimd.collective_compute(
                    "AllReduce", OP.add, replica_groups=cgroups,
                    ins=[st_b.opt()], outs=[st_o.opt()])
                stg1 = wsp.tile([1, GMAX], F32, tag="stg1", name="stg1")
                nc.sync.dma_start(out=stg1[:], in_=st_o[0:1, :])
                stg2 = wsp.tile([1, GMAX], F32, tag="stg2", name="stg2")
                nc.sync.dma_start(out=stg2[:], in_=st_o[1:2, :])
                mean = wsp.tile([1, GMAX], F32, tag="mean", name="mean")
                nc.vector.tensor_tensor(out=mean[:], in0=stg1[:],
                                        in1=invd[:], op=OP.mult)
                ex2 = wsp.tile([1, GMAX], F32, tag="ex2", name="ex2")
                nc.vector.tensor_tensor(out=ex2[:], in0=stg2[:],
                                        in1=invd[:], op=OP.mult)
                msq = wsp.tile([1, GMAX], F32, tag="msq", name="msq")
                nc.scalar.activation(out=msq[:], in_=mean[:], func=AF.Square)
                var = wsp.tile([1, GMAX], F32, tag="var", name="var")
                nc.vector.tensor_tensor(out=var[:], in0=ex2[:], in1=msq[:],
                                        op=OP.subtract)
                sd = wsp.tile([1, GMAX], F32, tag="sd", name="sd")
                nc.scalar.activation(out=sd[:], in_=var[:], func=AF.Sqrt,
                                     bias=epsc[0:1, 0:1])
                rstd = wsp.tile([1, GMAX], F32, tag="rstd", name="rstd")
                nc.vector.reciprocal(out=rstd[:], in_=sd[:])
                nmr2 = wsp.tile([1, GMAX], F32, tag="nmr2", name="nm2")
                nc.vector.tensor_tensor(out=nmr2[:], in0=mean[:], in1=rstd[:],
                                        op=OP.mult)
                nc.vector.tensor_scalar(out=nmr2[:], in0=nmr2[:], scalar1=-1.0,
                                        scalar2=None, op0=OP.mult)
                t1 = ps.tile([P, P], F32, tag="px", bufs=1, name="t1")
                nc.tensor.transpose(out=t1[0:GMAX, 0:1], in_=nmr2[:],
                                    identity=ident[0:1, 0:1])
                t2 = ps.tile([P, P], F32, tag="px", bufs=1, name="t2")
                nc.tensor.transpose(out=t2[0:GMAX, 0:1], in_=rstd[:],
                                    identity=ident[0:1, 0:1])
                nrcol = wsp.tile([GMAX, 2], BF, tag="nrcol", name="nrc")
                nc.vector.tensor_copy(out=nrcol[:, 0:1], in_=t1[0:GMAX, 0:1])
                nc.vector.tensor_copy(out=nrcol[:, 1:2], in_=t2[0:GMAX, 0:1])

                st_n, stage_n = make_stager(next_li)
                for w in range(NW):
                    mw = ps.tile([P, P], F32, tag="px", bufs=1, name="mw")
                    nc.tensor.matmul(out=mw[:, 0:2],
                                     lhsT=gmatT_s[:, w * P:(w + 1) * P],
                                     rhs=nrcol[:], start=True, stop=True)
                    mws = wsp.tile([P, 2], F32, tag="mws", name="mws")
                    nc.vector.tensor_copy(out=mws[:], in_=mw[:, 0:2])
                    xn = wsp.tile([P, P], F32, tag="xn", name="xn")
                    nc.scalar.activation(out=xn[:], in_=h_a[:, w, :],
                                         func=AF.Identity, scale=mws[:, 1:2],
                                         bias=mws[:, 0:1])
                    nc.vector.tensor_tensor(out=xn[:], in0=xn[:], in1=lnw,
                                            op=OP.mult)
                    nc.vector.tensor_tensor(out=xn[:], in0=xn[:], in1=lnb,
                                            op=OP.add)
                    # elu = max(x,0) + exp(min(x,0)) - 1
                    mn = wsp.tile([P, P], F32, tag="mn", name="mn")
                    nc.vector.tensor_scalar(out=mn[:], in0=xn[:], scalar1=0.0,
                                            scalar2=None, op0=OP.min)
                    nc.scalar.activation(out=mn[:], in_=mn[:], func=AF.Exp)
                    mx = wsp.tile([P, P], F32, tag="mx", name="mx")
                    nc.vector.tensor_scalar(out=mx[:], in0=xn[:], scalar1=0.0,
                                            scalar2=None, op0=OP.max)
                    nc.vector.tensor_tensor(out=mx[:], in0=mx[:], in1=mn[:],
                                            op=OP.add)
                    if add_resid:
                        nc.vector.tensor_scalar(out=mx[:], in0=mx[:],
                                                scalar1=1.0, scalar2=None,
                                                op0=OP.subtract)
                        rt = wsp.tile([P, P], BF, tag="rt", name="rt")
                        nc.sync.dma_start(out=rt[:],
                                          in_=rs_d[:, w * P:(w + 1) * P])
                        nc.vector.tensor_tensor(out=h_a[:, w, :], in0=mx[:],
                                                in1=rt[:], op=OP.add)
                    else:
                        nc.vector.tensor_scalar(out=h_a[:, w, :], in0=mx[:],
                                                scalar1=1.0, scalar2=None,
                                                op0=OP.subtract)
                    stage_n(w, ps)
            return st_n.get("xr_all")

        # ------------------------------------------------------------------
        def final_layer():
            wlf = wpk_s[:, 6 * P:6 * P + DF]
            wrf = wpk_s[:, 6 * P + DF:6 * P + 2 * DF]

            with tc.tile_pool(name="psf", bufs=1, space="PSUM") as ps:
                for g in groups:
                    gq, boff = grp_gathers(g)
                    for w in g["ws"]:
                        base = int(woff[w])
                        Tww = int(meta["Tw"][w])
                        xrfp = ps.tile([P, DF], F32, tag="ep", bufs=3,
                                       name="xrfp")
                        nc.tensor.matmul(out=xrfp[:], lhsT=hT[:, w, :],
                                         rhs=wrf, start=True, stop=True)
                        xrf = wsp.tile([P, DF], BF, tag="xrf", name="xrf")
                        nc.scalar.activation(out=xrf[:], in_=xrfp[:],
                                             func=AF.Identity)
                        ST = build_st(w, base, Tww)
                        ews2 = wsp.tile([2, TWMAX * P], BF, tag="ews2",
                                        name="ewsf")
                        nc.sync.dma_start(
                            out=ews2[0:2, :Tww * P],
                            in_=ew2_d[0:2, base * P:(base + Tww) * P])
                        cht = ps.tile([P, HEADS, P], F32, tag="cht",
                                      name="cht")
                        dnm = ps.tile([P, HEADS], F32, tag="fdnm", name="fdnm")
                        pend = []

                        def emit_cht(t0p, J, col0, lt0p, Bp, wqp, first,
                                     last, STx):
                            for j in range(J):
                                nc.tensor.matmul(
                                    out=cht[:].rearrange("p h c -> p (h c)"),
                                    lhsT=gq[:, col0 + j, :],
                                    rhs=Bp[:, j, :, :].rearrange(
                                        "p h c -> p (h c)"),
                                    start=(first and j == 0),
                                    stop=(last and j == J - 1))
                                nc.tensor.matmul(out=dnm[:],
                                                 lhsT=STx[:, :, lt0p + j],
                                                 rhs=wqp[:, j, :],
                                                 start=(first and j == 0),
                                                 stop=(last and j == J - 1))

                        lastsplit = max(si for si in range(S)
                                        if TS[si, w] > 0)
                        first = True
                        for si in range(S):
                            Th = int(TS[si, w])
                            if Th == 0:
                                continue
                            gq0 = g["smap"][si][w] + int(boff[si])
                            lt0 = int(run_off[si, w])
                            for t0p in range(0, Th, 2):
                                J = min(2, Th - t0p)
                                ea2 = wsp.tile([P, 2, DF], BF, tag="fea",
                                               bufs=3, name="fea")
                                for j in range(J):
                                    lt = lt0 + t0p + j
                                    col = gq0 + t0p + j
                                    gp = ps.tile([P, 2, P], BF, tag="pt",
                                                 bufs=2, name="gp")
                                    nc.tensor.transpose(out=gp[:, 0, :],
                                                        in_=gq[:, col, :],
                                                        identity=identb[:])
                                    nc.tensor.transpose(out=gp[:, 1, :],
                                                        in_=ST[:, :, lt],
                                                        identity=identb[:])
                                    gS = wsp.tile([P, 2, P], BF, tag="ghT",
                                                  bufs=3, name="ghT")
                                    nc.scalar.activation(out=gS[:],
                                                         in_=gp[:],
                                                         func=AF.Identity)
                                    ep = ps.tile([P, DF], F32, tag="ep",
                                                 bufs=3, name="fep")
                                    nc.tensor.matmul(
                                        out=ep[:],
                                        lhsT=ews2[0:2, lt * P:(lt + 1) * P],
                                        rhs=fwebb_s[0:2, :],
                                        start=True, stop=False)
                                    nc.tensor.matmul(out=ep[:],
                                                     lhsT=gS[:, 0, :],
                                                     rhs=wlf, start=False,
                                                     stop=False)
                                    nc.tensor.matmul(out=ep[:],
                                                     lhsT=gS[:, 1, :],
                                                     rhs=xrf[:], start=False,
                                                     stop=True)
                                    leaky(ea2[:, j, :], ep[:], [P, DF])
                                lg2 = wsp.tile([P, 2, DF], BF, tag="flg",
                                               bufs=3, name="flg")
                                nc.vector.tensor_tensor(
                                    out=lg2[:, :J, :], in0=ea2[:, :J, :],
                                    in1=attf_s[:, None, :].to_broadcast(
                                        [P, J, DF]),
                                    op=OP.mult)
                                lgr2 = wsp.tile([P, 2 * HEADS], BF,
                                                tag="flgr", name="flgr")
                                with nc.allow_low_precision(
                                        reason="bf16 head-logit reduce"):
                                    nc.vector.tensor_reduce(
                                        out=lgr2[:, :J * HEADS].rearrange(
                                            "p (j h) -> p j h", h=HEADS),
                                        in_=lg2[:, :J, :].rearrange(
                                            "p j (h c) -> p j h c", h=HEADS,
                                            c=P),
                                        axis=AX.X, op=OP.add)
                                wqp = wsp.tile([P, 2, HEADS], BF, tag="fwq",
                                               bufs=3, name="fwq")
                                nc.scalar.activation(
                                    out=wqp[:, :J, :],
                                    in_=lgr2[:, :J * HEADS].rearrange(
                                        "p (j h) -> p j h", h=HEADS),
                                    func=AF.Exp)
                                Bp = wsp.tile([P, 2, HEADS, P], BF, tag="fB",
                                              bufs=3, name="fB")
                                nc.vector.tensor_tensor(
                                    out=Bp[:, :J, :, :],
                                    in0=ST[:, :, lt0 + t0p:lt0 + t0p + J]
                                        .rearrange("p d j -> p j d")
                                        [:, :, None, :]
                                        .to_broadcast([P, J, HEADS, P]),
                                    in1=wqp[:, :J, :, None].to_broadcast(
                                        [P, J, HEADS, P]),
                                    op=OP.mult)
                                last = (si == lastsplit and
                                        t0p + J == Th)
                                if pend:
                                    emit_cht(*pend.pop())
                                pend.append((t0p, J, gq0 + t0p, lt0 + t0p,
                                             Bp, wqp, first, last, ST))
                                first = False
                        for args in pend:
                            emit_cht(*args)

                        # flush: nmr_h = ChT_h^T @ Wlf_h; out = bias +
                        #        mean_h numer/denom
                        chsb = wsp.tile([P, HEADS, P], BF, tag="chsb",
                                        name="chsb")
                        nc.scalar.activation(out=chsb[:], in_=cht[:],
                                             func=AF.Identity)
                        nmr = ps.tile([P, DF], F32, tag="ep", bufs=3,
                                      name="fnmr")
                        for h in range(HEADS):
                            nc.tensor.matmul(
                                out=nmr[:, h * P:(h + 1) * P],
                                lhsT=chsb[:, h, :],
                                rhs=wlf[:, h * P:(h + 1) * P],
                                start=True, stop=True)
                        rd = wsp.tile([P, HEADS], F32, tag="rd", name="frd")
                        nc.vector.tensor_scalar(out=rd[:], in0=dnm[:],
                                                scalar1=1e-16, scalar2=None,
                                                op0=OP.add)
                        nc.vector.reciprocal(out=rd[:], in_=rd[:])
                        nc.vector.tensor_scalar(out=rd[:], in0=rd[:],
                                                scalar1=1.0 / HEADS,
                                                scalar2=None, op0=OP.mult)
                        sc = wsp.tile([P, HEADS, P], F32, tag="sc", bufs=1,
                                      name="sc")
                        nc.vector.tensor_tensor(
                            out=sc[:],
                            in0=nmr[:].rearrange("p (h c) -> p h c", h=HEADS,
                                                 c=P),
                            in1=rd[:, :, None].to_broadcast([P, HEADS, P]),
                            op=OP.mult)
                        acc = wsp.tile([P, P], F32, tag="acc", name="acc")
                        nc.vector.tensor_reduce(
                            out=acc[:], in_=sc[:].rearrange("p h c -> p c h"),
                            axis=AX.X, op=OP.add)
                        nc.vector.tensor_tensor(out=acc[:], in0=acc[:],
                                                in1=fpk_s[:], op=OP.add)
                        nc.sync.dma_start(out=out_d[w * P:(w + 1) * P, :],
                                          in_=acc[:])

        # ---- the 4 layers (layer li+1's staging fused into li's P3) ----
        with tc.tile_pool(name="psS", bufs=1, space="PSUM") as psS:
            st0, stage0 = make_stager(0)
            for w in range(NW):
                stage0(w, psS)
        xr = st0["xr_all"]
        xr = hidden_layer(0, False, xr, 1)
        xr = hidden_layer(1, True, xr, 2)
        hidden_layer(2, False, xr, 3)
        final_layer()

    nc.compile()
    return nc


# ----------------------------------------------------------------------------
# Host-side driver
# ----------------------------------------------------------------------------

def _repP(v):
    v = np.asarray(v, np.float32).reshape(-1)
    return np.broadcast_to(v, (P, v.shape[0]))


def _winmaj(arr, lo_c, hi_c, NW):
    """[n, P] node-major slice -> [P, NW*P] window-major (padded)."""
    out = np.zeros((NW * P, P), np.float32)
    out[:hi_c - lo_c] = arr[lo_c:hi_c]
    return np.ascontiguousarray(
        out.reshape(NW, P, P).transpose(1, 0, 2).reshape(P, NW * P))


def make_in_maps(meta, inputs):
    NW, TT = meta["NW"], meta["TT"]
    lo, hi = meta["lo"], meta["hi"]
    x = np.asarray(inputs["x"], np.float32)
    resid = np.asarray(inputs["residual"], np.float32)
    ew = np.asarray(inputs["edge_weight"], np.float32)

    att = np.asarray(inputs["att"], np.float32)      # (3, H, C)
    attf = np.asarray(inputs["att_f"], np.float32)   # (H, DOUT)
    bl = np.asarray(inputs["bl"], np.float32)
    br = np.asarray(inputs["br"], np.float32)
    bias = np.asarray(inputs["bias"], np.float32)
    blf = np.asarray(inputs["bl_f"], np.float32)
    brf = np.asarray(inputs["br_f"], np.float32)
    biasf = np.asarray(inputs["bias_f"], np.float32)

    bf16 = ml_dtypes.bfloat16
    # hidden features stored (c h)-interleaved so the DVE alpha-weighting
    # multiply has a packed last dim (2x mode); PRM[c*H+h] = h*CH+c
    PRM = np.array([h * CH + c for c in range(CH) for h in range(HEADS)])
    wpk = np.concatenate(
        [np.asarray(inputs["Wl"], np.float32)[i][:, PRM] for i in range(3)]
        + [np.asarray(inputs["Wr"], np.float32)[i][:, PRM] for i in range(3)]
        + [np.asarray(inputs["Wl_f"], np.float32),
           np.asarray(inputs["Wr_f"], np.float32)], axis=1).astype(bf16)
    cpk = np.concatenate(
        [_repP(inputs["ln_w"][i]) for i in range(3)]
        + [_repP(inputs["ln_b"][i]) for i in range(3)]
        + [_repP(bias[i] + bl[i]) for i in range(3)], axis=1).astype(np.float32)
    apk = np.concatenate([_repP(att[i].reshape(-1)[PRM]) for i in range(3)],
                         axis=1).astype(bf16)
    We = np.asarray(inputs["We"], np.float32)
    webb = np.zeros((5, 12 * P), np.float32)
    for l in range(3):
        webb[0, l * 4 * P:(l + 1) * 4 * P] = np.tile((bl[l] + br[l])[PRM], 4)
        for r in range(4):
            webb[1 + r, l * 4 * P + r * P:l * 4 * P + (r + 1) * P] = \
                We[l].reshape(P)[PRM]
    fwebb = np.stack([blf + brf,
                      np.asarray(inputs["We_f"], np.float32).reshape(DF)])
    attf_rep = _repP(attf).astype(bf16)
    biaf_eff = biasf + blf.reshape(HEADS, -1).mean(axis=0)
    fpk = _repP(biaf_eff).astype(np.float32)

    common = dict(invd=meta["invd"].astype(np.float32), cpk=cpk, apk=apk,
                  wpk=wpk, attf=attf_rep, fpk=fpk,
                  webb=webb.astype(bf16), fwebb=fwebb.astype(bf16))

    S = meta["S"]
    woff, TS, run_off = meta["woff"], meta["TS"], meta["run_off"]
    nqcum, qoff, NQTOT = meta["nqcum"], meta["qoff"], meta["NQTOT"]

    in_maps = []
    for c in range(NCORES):
        ewc = np.zeros(TT * P, np.float32)
        m = meta["ewsl"][c] >= 0
        ewc[m] = ew[meta["ewsl"][c][m]]
        ew2 = np.stack([np.ones(TT * P, np.float32), ewc])
        ew5 = np.zeros((5, NQTOT * P), np.float32)
        ew5[0] = 1.0
        for w in range(NW):
            for si in range(S):
                Th = int(TS[si, w])
                lt0 = int(run_off[si, w])
                qq0 = int(qoff[w]) + int(nqcum[si, w])
                for q0 in range(0, Th, 4):
                    qq = qq0 + q0 // 4
                    for r in range(min(4, Th - q0)):
                        gt = int(woff[w]) + lt0 + q0 + r
                        ew5[1 + r, qq * P:(qq + 1) * P] = \
                            ewc[gt * P:(gt + 1) * P]
        in_maps.append(dict(
            h0s=_winmaj(x, lo[c], hi[c], NW).astype(bf16),
            rs=_winmaj(resid, lo[c], hi[c], NW).astype(bf16),
            dsti=meta["dsti"][c].astype(bf16),
            idx=np.tile(meta["idxm"][c], (8, 1)),
            ew5=ew5.astype(bf16),
            ew2=ew2.astype(bf16),
            gmat=meta["gmat"][c].astype(bf16),
            gmatT=meta["gmatT"][c].astype(bf16),
            **common))
    return in_maps


def assemble(meta, results):
    N = meta["N"]
    lo, hi = meta["lo"], meta["hi"]
    out = np.zeros((N, P), np.float32)
    for c in range(NCORES):
        n = int(hi[c] - lo[c])
        out[lo[c]:hi[c]] = results[c]["out"][:n]
    return out


_CACHE = {}


def kernel(**inputs):
    ei = np.asarray(inputs["edge_index"])
    bt = np.asarray(inputs["batch"])
    key = (ei.shape, bt.shape, hash(ei.tobytes()), hash(bt.tobytes()))
    if key not in _CACHE:
        meta = build_meta(ei, bt)
        nc = build_program(meta)
        _CACHE[key] = (meta, nc)
    meta, nc = _CACHE[key]
    in_maps = make_in_maps(meta, inputs)
    res = run_bass_kernel_spmd(nc, in_maps, list(range(NCORES)))
    return assemble(meta, res.results)



# revision 30
# speedup vs baseline: 1.0476x; 1.0473x over previous
"""GATv2 backbone (4 layers) on 8 Trainium2 NeuronCores.

Strategy (v4):
  * v3 + cross-layer phase fusion: each layer's normalize (P3) stages the
    NEXT layer's tables per window (transpose + xl/xr projection + DRAM
    bounce) and fires the per-band AllGathers early, removing the serial
    P3->P0 boundary. Gathers issue with single_packet=False (descriptors
    spread over all 16 SDMA engines); deeper tile buffering (gq bufs=3,
    per-quad chain bufs=4) keeps more quads in flight across the
    10-step cross-engine dependency chain.

Strategy (v3):
  * Nodes partitioned into 8 contiguous ranges (edge-balanced); edges owned
    by the dst core, grouped by 128-node dst windows. Windows are divided
    into SPLITS contiguous bands; each band's node table (<=32767 rows, so
    int16 gather indices work) is AllGathered separately, and the AllGathers
    are issued as soon as their band's xl shard is staged -- they pipeline
    with P0 and with edge-phase compute of earlier bands.
  * Gathers: nc.gpsimd.dma_gather, <=8 tiles (1024 rows) per op, one op per
    (window-group, band, chunk) -- ~130 ops/layer vs 850 indirect DMAs.
  * Hidden layers gather xl rows (256B); the final layer gathers h rows
    (256B) and computes xlf = h @ Wlf on-chip (4x less gather + AllGather
    traffic), with the numerator factored as (B_h^T @ gh) @ Wlf_h.
  * One-hot ST built bf16 in [e, d, t] layout (packed last dim -> DVE 2x
    mode); S = PE transpose of ST slices, PSUM->SBUF copies on the ACT
    engine. Edge-weight rank-1s and both linear biases fold into a single
    [ones; ew-rows] @ [bias; We-blocks] matmul per quad; numerator bias
    recovered via sum(alpha)=1. nmr|dnm share one PSUM accumulation matmul.
  * leaky_relu via Prelu (same ACT table set as Exp/Square -> no reloads).
"""

import contextlib
import os

import ml_dtypes
import numpy as np

from concourse import bass, bacc, mybir, tile
from concourse.bass_utils import run_bass_kernel_spmd
from concourse.masks import make_identity

P = 128
NCORES = 8
GMAX = 50
HEADS = 4
DHID = 128
CH = DHID // HEADS          # 32
DF = 512
NEG = 0.2
EPS = 1e-5
GW = 1                      # windows per gather group
MAXT = 8                    # tiles per dma_gather op (1024 idxs, HW limit)
NSPLIT = 4                  # table bands (pipelined AllGathers)
NQUEUES = 4                 # SWDGE queues for gather DMAs
USE_LRELU = os.environ.get("K_NO_PRELU", "") != "1"

F32 = mybir.dt.float32
BF = mybir.dt.bfloat16
I32 = mybir.dt.int32
I16 = mybir.dt.int16
AX = mybir.AxisListType
OP = mybir.AluOpType
AF = mybir.ActivationFunctionType


# ----------------------------------------------------------------------------
# Host preprocessing: graph partitioning + static schedule
# ----------------------------------------------------------------------------

def build_meta(edge_index, batch):
    N = batch.shape[0]
    E = edge_index.shape[1]
    src = np.asarray(edge_index[0], dtype=np.int64)
    dst = np.asarray(edge_index[1], dtype=np.int64)
    batch = np.asarray(batch, dtype=np.int64)

    deg = np.bincount(dst, minlength=N)
    cum = np.concatenate([[0], np.cumsum(deg)])

    bounds = [0]
    for c in range(1, NCORES):
        n = int(np.searchsorted(cum, c * E / NCORES))
        bounds.append(min(max(n, bounds[-1] + 1), N - (NCORES - c)))
    bounds.append(N)
    lo = np.array(bounds[:-1])
    hi = np.array(bounds[1:])

    NW = int(max((hi - lo + P - 1) // P))
    NPAD = NW * P
    S = min(NSPLIT, NW)
    swin = [a for a in np.array_split(np.arange(NW), S)]
    sbase = np.array([int(a[0]) for a in swin])
    scount = np.array([len(a) for a in swin])
    TBLROWS = NCORES * scount * P
    assert (TBLROWS <= 32767).all(), TBLROWS
    split_id = np.zeros(NW, np.int64)
    for si, a in enumerate(swin):
        split_id[a] = si

    core_of = np.repeat(np.arange(NCORES), (hi - lo))
    off = np.arange(N) - lo[core_of]
    wof = off // P                       # window-within-core (may pad-overflow
    wof = np.minimum(wof, NW - 1)        # never: off < NPAD)
    nsp = split_id[wof]
    nrow = core_of * scount[nsp] * P + (off - sbase[nsp] * P)

    order = np.argsort(dst, kind="stable")

    cnt = np.zeros((S, NCORES, NW), np.int64)
    lists = {}
    for c in range(NCORES):
        for w in range(NW):
            a = lo[c] + w * P
            b = min(a + P, hi[c])
            if a >= b:
                lists[(c, w)] = [np.empty(0, np.int64)] * S
                continue
            ids = order[int(cum[a]):int(cum[b])]
            sp = nsp[src[ids]]
            per = []
            for si in range(S):
                idsS = ids[sp == si]
                idsS = idsS[np.argsort(nrow[src[idsS]], kind="stable")]
                per.append(idsS)
                cnt[si, c, w] = len(idsS)
            lists[(c, w)] = per

    TS = ((cnt.max(axis=1) + P - 1) // P).astype(np.int64)    # [S, NW]
    for w in range(NW):
        if TS[:, w].sum() == 0:
            TS[0, w] = 1
    run_off = np.zeros((S, NW), np.int64)
    for si in range(1, S):
        run_off[si] = run_off[si - 1] + TS[si - 1]
    Tw = TS.sum(axis=0)
    woff = np.concatenate([[0], np.cumsum(Tw)]).astype(np.int64)
    TT = int(woff[-1])

    dsti = np.full((NCORES, P, TT), -1, np.int64)
    ewsl = np.full((NCORES, TT * P), -1, np.int64)
    srow = np.zeros((NCORES, TT * P), np.int64)
    for c in range(NCORES):
        for w in range(NW):
            for si in range(S):
                ids_h = lists[(c, w)][si]
                n = len(ids_h)
                if n == 0:
                    continue
                tb = int(woff[w] + run_off[si, w])
                slot = np.arange(n)
                tt = tb + slot // P
                pp = slot % P
                dsti[c, pp, tt] = dst[ids_h] - (lo[c] + w * P)
                flat = tt * P + pp
                ewsl[c, flat] = ids_h
                srow[c, flat] = nrow[src[ids_h]]

    # per-window quad schedule (band-major)
    nqS = ((TS + 3) // 4).astype(np.int64)            # [S, NW]
    nqcum = np.zeros((S, NW), np.int64)
    for si in range(1, S):
        nqcum[si] = nqcum[si - 1] + nqS[si - 1]
    nquad = nqS.sum(axis=0)
    NQBMAX = int(nqS.max())
    qoff = np.concatenate([[0], np.cumsum(nquad)]).astype(np.int64)
    NQTOT = int(qoff[-1])
    NQMAX = int(nquad.max())

    # gather-op schedule (static, identical across cores)
    groups = []
    icols = 0
    for g0 in range(0, NW, GW):
        ws = list(range(g0, min(g0 + GW, NW)))
        smap = [dict() for _ in range(S)]
        tgb = []
        ops = []
        for si in range(S):
            run = 0
            for w in ws:
                smap[si][w] = run
                run += int(TS[si, w])
            tgb.append(run)
            tiles = []
            for w in ws:
                for t in range(int(TS[si, w])):
                    gt = int(woff[w] + run_off[si, w]) + t
                    tiles.append((gt, smap[si][w] + t))
            for i in range(0, len(tiles), MAXT):
                ch = tiles[i:i + MAXT]
                ops.append(dict(split=si, coff=icols, nt=len(ch),
                                gq0=ch[0][1], gtiles=[x[0] for x in ch]))
                icols += len(ch) * 8
        groups.append(dict(ws=ws, tgb=tgb, smap=smap, ops=ops))
    ICOLS = icols
    TGMAX = max(max(g["tgb"]) for g in groups)
    TGFULL = max(sum(g["tgb"]) for g in groups)
    TWMAX = int(Tw.max())
    fnz = np.array([min(si for si in range(S) if TS[si, w] > 0)
                    for w in range(NW)])
    lnz = np.array([max(si for si in range(S) if TS[si, w] > 0)
                    for w in range(NW)])

    idxm = np.zeros((NCORES, 16, ICOLS), np.int16)
    for c in range(NCORES):
        for g in groups:
            for op in g["ops"]:
                nt = op["nt"]
                vals = np.zeros(nt * P, np.int64)
                for j, gt in enumerate(op["gtiles"]):
                    vals[j * P:(j + 1) * P] = srow[c, gt * P:(gt + 1) * P]
                idxm[c, :, op["coff"]:op["coff"] + nt * 8] = (
                    vals.reshape(nt * 8, 16).T.astype(np.int16))

    gmat = np.zeros((NCORES, P, NW * GMAX), np.float32)
    gmatT = np.zeros((NCORES, GMAX, NW * P), np.float32)
    for c in range(NCORES):
        nreal = int(hi[c] - lo[c])
        g = batch[lo[c]:hi[c]]
        r = np.arange(nreal)
        gmat[c, r % P, (r // P) * GMAX + g] = 1.0
        gmatT[c, g, (r // P) * P + (r % P)] = 1.0

    cntg = np.bincount(batch, minlength=GMAX).astype(np.float32)
    invd = (1.0 / (np.maximum(cntg, 1.0) * DHID)).reshape(1, GMAX)

    return dict(N=N, E=E, NW=NW, NPAD=NPAD, S=S, swin=swin, sbase=sbase,
                scount=scount, TBLROWS=TBLROWS, TT=TT, ICOLS=ICOLS,
                TGMAX=TGMAX, TWMAX=TWMAX, TS=TS, run_off=run_off, Tw=Tw,
                woff=woff, lo=lo, hi=hi, nqS=nqS, nqcum=nqcum, nquad=nquad,
                qoff=qoff, NQTOT=NQTOT, NQMAX=NQMAX, NQBMAX=NQBMAX,
                TGFULL=TGFULL,
                groups=groups, fnz=fnz, lnz=lnz,
                dsti=dsti, ewsl=ewsl, idxm=idxm, gmat=gmat, gmatT=gmatT,
                invd=invd)


# ----------------------------------------------------------------------------
# Bass program
# ----------------------------------------------------------------------------

def build_program(meta):
    NW, NPAD, TT = meta["NW"], meta["NPAD"], meta["TT"]
    S, sbase, scount = meta["S"], meta["sbase"], meta["scount"]
    TBLROWS, ICOLS = meta["TBLROWS"], meta["ICOLS"]
    TGMAX, TWMAX = meta["TGMAX"], meta["TWMAX"]
    TS, run_off, woff = meta["TS"], meta["run_off"], meta["woff"]
    nqS, nqcum, nquad, qoff = (meta["nqS"], meta["nqcum"], meta["nquad"],
                               meta["qoff"])
    NQTOT, NQMAX = meta["NQTOT"], meta["NQMAX"]
    NQBMAX = meta["NQBMAX"]
    TGFULL = meta["TGFULL"]
    groups = meta["groups"]
    swin = meta["swin"]
    fnz, lnz = meta["fnz"], meta["lnz"]

    nc = bacc.Bacc("TRN2", target_bir_lowering=False, debug=False,
                   enable_asserts=False, num_devices=NCORES,
                   num_swdge_queues=1)

    h0s_d = nc.dram_tensor("h0s", [P, NW * P], BF, kind="ExternalInput")
    rs_d = nc.dram_tensor("rs", [P, NW * P], BF, kind="ExternalInput")
    dsti_d = nc.dram_tensor("dsti", [P, TT], BF, kind="ExternalInput")
    idx_d = nc.dram_tensor("idx", [P, ICOLS], I16, kind="ExternalInput")
    ew5_d = nc.dram_tensor("ew5", [5, NQTOT * P], BF, kind="ExternalInput")
    ew2_d = nc.dram_tensor("ew2", [2, TT * P], BF, kind="ExternalInput")
    gmat_d = nc.dram_tensor("gmat", [P, NW * GMAX], BF, kind="ExternalInput")
    gmatT_d = nc.dram_tensor("gmatT", [GMAX, NW * P], BF, kind="ExternalInput")
    invd_d = nc.dram_tensor("invd", [1, GMAX], F32, kind="ExternalInput")
    cpk_d = nc.dram_tensor("cpk", [P, 9 * P], F32, kind="ExternalInput")
    apk_d = nc.dram_tensor("apk", [P, 3 * P], BF, kind="ExternalInput")
    wpk_d = nc.dram_tensor("wpk", [P, 6 * P + 2 * DF], BF, kind="ExternalInput")
    attf_d = nc.dram_tensor("attf", [P, DF], BF, kind="ExternalInput")
    fpk_d = nc.dram_tensor("fpk", [P, P], F32, kind="ExternalInput")
    webb_d = nc.dram_tensor("webb", [5, 12 * P], BF, kind="ExternalInput")
    fwebb_d = nc.dram_tensor("fwebb", [2, DF], BF, kind="ExternalInput")
    out_d = nc.dram_tensor("out", [NPAD, P], F32, kind="ExternalOutput")

    with tile.TileContext(nc) as tc, contextlib.ExitStack() as ctx:
        dram = ctx.enter_context(tc.tile_pool(name="dram", bufs=1, space="DRAM"))
        cst = ctx.enter_context(tc.tile_pool(name="cst", bufs=1))
        per = ctx.enter_context(tc.tile_pool(name="per", bufs=1))
        wsp = ctx.enter_context(tc.tile_pool(name="wsp", bufs=2))
        gpo = ctx.enter_context(tc.tile_pool(name="gpo", bufs=2))

        xl_b = dram.tile([NPAD, P], BF)
        tbl = [nc.dram_tensor(f"tbl{si}", [int(TBLROWS[si]), P], BF,
                              kind="Internal", addr_space="Shared")
               for si in range(S)]
        st_b = dram.tile([2, GMAX], F32)
        st_o = dram.tile([2, GMAX], F32)
        cgroups = [list(range(NCORES))]

        # --- constants / residents ---
        ident = cst.tile([P, P], F32)
        make_identity(nc, ident[:])
        identb = cst.tile([P, P], BF)
        nc.vector.tensor_copy(out=identb[:], in_=ident[:])
        iota_row = cst.tile([P, P], I32)
        nc.gpsimd.iota(iota_row[:], pattern=[[1, P]], base=0,
                       channel_multiplier=0)
        iota_rowb = cst.tile([P, P], BF)
        nc.vector.tensor_copy(out=iota_rowb[:], in_=iota_row[:])
        iota3 = cst.tile([P, P, TWMAX], BF)
        nc.vector.tensor_copy(
            out=iota3[:],
            in_=iota_rowb[:, :, None].to_broadcast([P, P, TWMAX]))
        epsc = cst.tile([P, 1], F32)
        nc.vector.memset(epsc[:], EPS)
        invd = cst.tile([1, GMAX], F32)
        nc.sync.dma_start(out=invd[:], in_=invd_d[:, :])
        wpk_s = cst.tile([P, 6 * P + 2 * DF], BF)
        nc.sync.dma_start(out=wpk_s[:], in_=wpk_d[:, :])
        dsti_s = cst.tile([P, TT], BF)
        nc.sync.dma_start(out=dsti_s[:], in_=dsti_d[:, :])
        idx_s = cst.tile([P, ICOLS], I16)
        nc.sync.dma_start(out=idx_s[:], in_=idx_d[:, :])
        gmat_s = cst.tile([P, NW * GMAX], BF)
        nc.sync.dma_start(out=gmat_s[:], in_=gmat_d[:, :])
        gmatT_s = cst.tile([GMAX, NW * P], BF)
        nc.sync.dma_start(out=gmatT_s[:], in_=gmatT_d[:, :])
        cpk_s = cst.tile([P, 9 * P], F32)
        nc.sync.dma_start(out=cpk_s[:], in_=cpk_d[:, :])
        apk_s = cst.tile([P, 3 * P], BF)
        nc.sync.dma_start(out=apk_s[:], in_=apk_d[:, :])
        attf_s = cst.tile([P, DF], BF)
        nc.sync.dma_start(out=attf_s[:], in_=attf_d[:, :])
        fpk_s = cst.tile([P, P], F32)
        nc.sync.dma_start(out=fpk_s[:], in_=fpk_d[:, :])
        webb_s = cst.tile([5, 12 * P], BF)
        nc.sync.dma_start(out=webb_s[:], in_=webb_d[:, :])
        fwebb_s = cst.tile([2, DF], BF)
        nc.sync.dma_start(out=fwebb_s[:], in_=fwebb_d[:, :])

        h_a = per.tile([P, NW, P], BF, tag="h_a")
        hT = per.tile([P, NW, P], BF, tag="hT")
        nc.sync.dma_start(out=h_a[:, :, :],
                          in_=h0s_d[:, :].rearrange("p (w f) -> p w f", w=NW))

        def leaky(dst_ap, src_ap, shape):
            if USE_LRELU:
                # Prelu == leaky relu with param alpha; unlike Lrelu it is in
                # the same ACT table set as Exp/Square -> no table reloads.
                nc.scalar.activation(out=dst_ap, in_=src_ap, func=AF.Prelu,
                                     alpha=NEG)
            else:
                r = wsp.tile(shape, F32, tag="lrtmp", bufs=1, name="lr")
                rr = r[tuple(slice(0, s) for s in dst_ap.shape)]
                nc.scalar.activation(out=rr, in_=src_ap, func=AF.Relu,
                                     scale=-(1.0 - NEG))
                nc.vector.tensor_tensor(out=dst_ap, in0=src_ap, in1=rr,
                                        op=OP.add)

        split_last = {int(a[-1]): si for si, a in enumerate(swin)}
        SGW = 2                 # staging chunk (windows per xl_b DMA)
        breaks = set(range(SGW - 1, NW, SGW)) | set(split_last) | {NW - 1}

        def emit_ag(si):
            a = int(sbase[si]) * P
            b = a + int(scount[si]) * P
            nc.gpsimd.collective_compute(
                "AllGather", OP.bypass, replica_groups=cgroups,
                ins=[xl_b[a:b, :].opt()], outs=[tbl[si][:, :].opt()])

        qctr = [0]

        def grp_gathers(g, band=None):
            if band is not None:
                gq = gpo.tile([P, TGMAX, P], BF, tag="gq", name="gq",
                              bufs=5)
            else:
                gq = gpo.tile([P, TGFULL, P], BF, tag="gqf", name="gqf",
                              bufs=3)
            boff = np.concatenate([[0], np.cumsum(g["tgb"])]).astype(int)
            for op in g["ops"]:
                if band is not None and op["split"] != band:
                    continue
                g0 = op["gq0"] + (0 if band is not None
                                  else int(boff[op["split"]]))
                nc.gpsimd.dma_gather(
                    gq[:, g0:g0 + op["nt"], :],
                    tbl[op["split"]][:, :],
                    idx_s[:, op["coff"]:op["coff"] + op["nt"] * 8],
                    op["nt"] * P, op["nt"] * P, P,
                    single_packet=False)
            return gq, boff

        def build_st(w, t0, Tn):
            """ST2[e, d, t] one-hot (bf16, packed last dim -> DVE 2x)."""
            ST = wsp.tile([P, P, TWMAX], BF, tag="ST", name="ST")
            nc.vector.tensor_tensor(
                out=ST[:, :, :Tn],
                in0=iota3[:, :, :Tn],
                in1=dsti_s[:, None, t0:t0 + Tn]
                    .to_broadcast([P, P, Tn]),
                op=OP.is_equal)
            return ST

        # ------------------------------------------------------------------
        def make_stager(li):
            """Staging for layer li's tables: transpose h, project (hidden)
            or copy (final), bounce to DRAM, fire per-band AllGathers.
            Called per window, fused into the previous layer's P3."""
            st = dict(run=False, w0=0, xsg=None)
            if li < 3:
                st["xr_all"] = wsp.tile([P, NW, P], BF, tag="xra", bufs=1,
                                        name="xra")
                wl = wpk_s[:, li * P:(li + 1) * P]
                wr = wpk_s[:, (3 + li) * P:(4 + li) * P]

            def stage(w, ps):
                nc.sync.dma_start(out=hT[:, w, :], in_=h_a[:, w, :],
                                  transpose=True)
                if not st["run"]:
                    st["run"] = True
                    st["w0"] = w
                    if li < 3:
                        st["xsg"] = wsp.tile([P, SGW, P], BF, tag="xsg",
                                             name="xsg")
                w0 = st["w0"]
                if li < 3:
                    xp = ps.tile([P, P], F32, tag="px", bufs=2, name="px")
                    nc.tensor.matmul(out=xp[:], lhsT=hT[:, w, :], rhs=wl,
                                     start=True, stop=True)
                    nc.scalar.activation(out=st["xsg"][:, w - w0, :],
                                         in_=xp[:], func=AF.Identity)
                    xrp = ps.tile([P, P], F32, tag="px", bufs=2, name="xrp")
                    nc.tensor.matmul(out=xrp[:], lhsT=hT[:, w, :], rhs=wr,
                                     start=True, stop=True)
                    nc.scalar.activation(out=st["xr_all"][:, w, :],
                                         in_=xrp[:], func=AF.Identity)
                    if w in breaks:
                        nc.sync.dma_start(
                            out=xl_b[w0 * P:(w + 1) * P, :].rearrange(
                                "(w p) f -> p w f", p=P),
                            in_=st["xsg"][:, :w - w0 + 1, :])
                        st["run"] = False
                else:
                    if w in breaks:
                        nc.sync.dma_start(
                            out=xl_b[w0 * P:(w + 1) * P, :].rearrange(
                                "(w p) f -> p w f", p=P),
                            in_=h_a[:, w0:w + 1, :])
                        st["run"] = False
                if w in split_last:
                    emit_ag(split_last[w])
            return st, stage

        def hidden_layer(li, add_resid, xr_all, next_li):
            attr = apk_s[:, li * P:(li + 1) * P]
            lnw = cpk_s[:, li * P:(li + 1) * P]
            lnb = cpk_s[:, (3 + li) * P:(4 + li) * P]
            bia = cpk_s[:, (6 + li) * P:(7 + li) * P]

            with tc.tile_pool(name=f"ps{li}", bufs=1, space="PSUM") as ps:
                # P2: edge pipeline, band-major so AllGather si+1 overlaps
                # band-si compute; per-window numerators accumulate in SBUF.
                nd_all = wsp.tile([P, NW, P + HEADS], BF, tag="nda",
                                  bufs=1, name="nda")
                for band in range(S):
                    for g in groups:
                        if all(TS[band, w] == 0 for w in g["ws"]):
                            continue
                        gq, _ = grp_gathers(g, band)
                        gqv = gq[:].rearrange("p t (c h) -> p t c h",
                                              h=HEADS, c=CH)
                        for w in g["ws"]:
                            Th = int(TS[band, w])
                            if Th == 0:
                                continue
                            t0g = int(woff[w] + run_off[band, w])
                            ST = build_st(w, t0g, Th)
                            nqb = (Th + 3) // 4
                            qb0 = int(qoff[w] + nqcum[band, w])
                            ews5 = wsp.tile([5, NQBMAX * P], BF, tag="ews",
                                            bufs=4, name="ews")
                            nc.sync.dma_start(
                                out=ews5[0:5, :nqb * P],
                                in_=ew5_d[0:5, qb0 * P:(qb0 + nqb) * P])
                            nd = ps.tile([P, P + HEADS], F32, tag="nd",
                                         name="nd")
                            gq0 = g["smap"][band][w]

                            def emit_nd(q0, Q, mmw):
                                for t in range(Q):
                                    nc.tensor.matmul(
                                        out=nd[:], lhsT=ST[:, :, q0 + t],
                                        rhs=mmw[:, t, :],
                                        start=(q0 + t == 0),
                                        stop=(q0 + t == Th - 1))

                            pend = None
                            for q0 in range(0, Th, 4):
                                Q = min(4, Th - q0)
                                qq = q0 // 4
                                Ssb = wsp.tile([P, 4, P], BF, tag="ssb",
                                               bufs=5, name="ssb")
                                sp = ps.tile([P, 4, P], BF, tag="pt",
                                             bufs=2, name="sp")
                                for t in range(Q):
                                    nc.tensor.transpose(
                                        out=sp[:, t, :],
                                        in_=ST[:, :, q0 + t],
                                        identity=identb[:])
                                nc.scalar.activation(out=Ssb[:, :Q, :],
                                                     in_=sp[:, :Q, :],
                                                     func=AF.Identity)
                                ep = ps.tile([P, 4 * P], F32, tag="ep",
                                             bufs=2, name="ep")
                                # ef + biases first (host data, always ready)
                                # and the gather-dependent copy last, so the
                                # in-order PE queue head never parks on a
                                # not-yet-landed DMA.
                                nc.tensor.matmul(
                                    out=ep[:, :Q * P],
                                    lhsT=ews5[0:Q + 1, qq * P:(qq + 1) * P],
                                    rhs=webb_s[0:Q + 1,
                                               li * 4 * P:li * 4 * P + Q * P],
                                    start=True, stop=False)
                                for t in range(Q):
                                    blk = ep[:, t * P:(t + 1) * P]
                                    nc.tensor.matmul(out=blk,
                                                     lhsT=Ssb[:, t, :],
                                                     rhs=xr_all[:, w, :],
                                                     start=False, stop=False)
                                nc.tensor.matmul(
                                    out=ep[:, :Q * P], lhsT=identb[:],
                                    rhs=gq[:, gq0 + q0:gq0 + q0 + Q, :],
                                    start=False, stop=True)
                                ea = wsp.tile([P, 4 * P], BF, tag="ea", bufs=5,
                                              name="ea")
                                leaky(ea[:, :Q * P], ep[:, :Q * P], [P, 4 * P])
                                lg = wsp.tile([P, 4 * P], BF, tag="lg", bufs=5,
                                              name="lg")
                                nc.vector.tensor_tensor(
                                    out=lg[:, :Q * P], in0=ea[:, :Q * P],
                                    in1=attr[:, None, :].to_broadcast(
                                        [P, Q, P]),
                                    op=OP.mult)
                                lgr = wsp.tile([P, 4 * HEADS], F32, tag="lgr",
                                               bufs=4, name="lgr")
                                nc.vector.tensor_reduce(
                                    out=lgr[:, :Q * HEADS].rearrange(
                                        "p (t h) -> p t h", h=HEADS),
                                    in_=lg[:].rearrange(
                                        "p (t c h) -> p t h c", h=HEADS,
                                        c=CH)[:, :Q, :, :],
                                    axis=AX.X, op=OP.add)
                                mmw = wsp.tile([P, 4, P + HEADS], BF,
                                               tag="mm", bufs=4, name="mm")
                                nc.scalar.activation(
                                    out=mmw[:, :Q, P:P + HEADS],
                                    in_=lgr[:, :Q * HEADS].rearrange(
                                        "p (t h) -> p t h", h=HEADS),
                                    func=AF.Exp)
                                nc.vector.tensor_tensor(
                                    out=mmw[:, :Q, 0:P].rearrange(
                                        "p q (c h) -> p q c h", h=HEADS,
                                        c=CH),
                                    in0=gqv[:, gq0 + q0:gq0 + q0 + Q, :, :],
                                    in1=mmw[:, :Q, None, P:P + HEADS]
                                        .to_broadcast([P, Q, CH, HEADS]),
                                    op=OP.mult)
                                if pend is not None:
                                    emit_nd(*pend)
                                pend = (q0, Q, mmw)
                            if pend is not None:
                                emit_nd(*pend)
                            if band == int(fnz[w]):
                                nc.scalar.activation(out=nd_all[:, w, :],
                                                     in_=nd[:],
                                                     func=AF.Identity)
                            else:
                                nc.vector.tensor_tensor(
                                    out=nd_all[:, w, :], in0=nd_all[:, w, :],
                                    in1=nd[:], op=OP.add)

                # window flush + LN stats
                stp = ps.tile([2, GMAX], F32, tag="stats", name="stp")
                for w in range(NW):
                    rd = wsp.tile([P, HEADS], F32, tag="rd", name="rd")
                    nc.vector.tensor_scalar(out=rd[:],
                                            in0=nd_all[:, w, P:P + HEADS],
                                            scalar1=1e-16, scalar2=None,
                                            op0=OP.add)
                    nc.vector.reciprocal(out=rd[:], in_=rd[:])
                    oT = wsp.tile([P, HEADS, CH], F32, tag="oT", name="oT")
                    nc.vector.tensor_tensor(
                        out=oT[:],
                        in0=nd_all[:, w, :P].rearrange("p (c h) -> p h c",
                                                       h=HEADS, c=CH),
                        in1=rd[:, :, None].to_broadcast([P, HEADS, CH]),
                        op=OP.mult)
                    nc.vector.tensor_tensor(
                        out=h_a[:, w, :],
                        in0=oT[:].rearrange("p h c -> p (h c)"),
                        in1=bia, op=OP.add)
                    s12 = wsp.tile([P, 2], F32, tag="s12", name="s12")
                    nc.vector.tensor_reduce(out=s12[:, 0:1],
                                            in_=h_a[:, w, :],
                                            axis=AX.X, op=OP.add)
                    sqj = wsp.tile([P, P], BF, tag="sqj", name="sqj")
                    nc.scalar.activation(out=sqj[:], in_=h_a[:, w, :],
                                         func=AF.Square,
                                         accum_out=s12[:, 1:2])
                    s12b = wsp.tile([P, 2], BF, tag="s12b", name="s12b")
                    nc.vector.tensor_copy(out=s12b[:], in_=s12[:])
                    nc.tensor.matmul(
                        out=stp[:, :], lhsT=s12b[:],
                        rhs=gmat_s[:, w * GMAX:(w + 1) * GMAX],
                        start=(w == 0), stop=(w == NW - 1))

                # P3: stats -> mean/rstd -> normalize + elu
                sts = wsp.tile([2, GMAX], F32, tag="sts", name="sts")
                nc.vector.tensor_copy(out=sts[:], in_=stp[:])
                nc.sync.dma_start(out=st_b[:, :], in_=sts[:])
                nc.gpsimd.collective_compute(
                    "AllReduce", OP.add, replica_groups=cgroups,
                    ins=[st_b.opt()], outs=[st_o.opt()])
                stg1 = wsp.tile([1, GMAX], F32, tag="stg1", name="stg1")
                nc.sync.dma_start(out=stg1[:], in_=st_o[0:1, :])
                stg2 = wsp.tile([1, GMAX], F32, tag="stg2", name="stg2")
                nc.sync.dma_start(out=stg2[:], in_=st_o[1:2, :])
                mean = wsp.tile([1, GMAX], F32, tag="mean", name="mean")
                nc.vector.tensor_tensor(out=mean[:], in0=stg1[:],
                                        in1=invd[:], op=OP.mult)
                ex2 = wsp.tile([1, GMAX], F32, tag="ex2", name="ex2")
                nc.vector.tensor_tensor(out=ex2[:], in0=stg2[:],
                                        in1=invd[:], op=OP.mult)
                msq = wsp.tile([1, GMAX], F32, tag="msq", name="msq")
                nc.scalar.activation(out=msq[:], in_=mean[:], func=AF.Square)
                var = wsp.tile([1, GMAX], F32, tag="var", name="var")
                nc.vector.tensor_tensor(out=var[:], in0=ex2[:], in1=msq[:],
                                        op=OP.subtract)
                sd = wsp.tile([1, GMAX], F32, tag="sd", name="sd")
                nc.scalar.activation(out=sd[:], in_=var[:], func=AF.Sqrt,
                                     bias=epsc[0:1, 0:1])
                rstd = wsp.tile([1, GMAX], F32, tag="rstd", name="rstd")
                nc.vector.reciprocal(out=rstd[:], in_=sd[:])
                nmr2 = wsp.tile([1, GMAX], F32, tag="nmr2", name="nm2")
                nc.vector.tensor_tensor(out=nmr2[:], in0=mean[:], in1=rstd[:],
                                        op=OP.mult)
                nc.vector.tensor_scalar(out=nmr2[:], in0=nmr2[:], scalar1=-1.0,
                                        scalar2=None, op0=OP.mult)
                t1 = ps.tile([P, P], F32, tag="px", bufs=2, name="t1")
                nc.tensor.transpose(out=t1[0:GMAX, 0:1], in_=nmr2[:],
                                    identity=ident[0:1, 0:1])
                t2 = ps.tile([P, P], F32, tag="px", bufs=2, name="t2")
                nc.tensor.transpose(out=t2[0:GMAX, 0:1], in_=rstd[:],
                                    identity=ident[0:1, 0:1])
                nrcol = wsp.tile([GMAX, 2], BF, tag="nrcol", name="nrc")
                nc.vector.tensor_copy(out=nrcol[:, 0:1], in_=t1[0:GMAX, 0:1])
                nc.vector.tensor_copy(out=nrcol[:, 1:2], in_=t2[0:GMAX, 0:1])

                st_n, stage_n = make_stager(next_li)
                for w in range(NW):
                    mw = ps.tile([P, P], F32, tag="stats", bufs=1, name="mw")
                    nc.tensor.matmul(out=mw[:, 0:2],
                                     lhsT=gmatT_s[:, w * P:(w + 1) * P],
                                     rhs=nrcol[:], start=True, stop=True)
                    mws = wsp.tile([P, 2], F32, tag="mws", name="mws")
                    nc.vector.tensor_copy(out=mws[:], in_=mw[:, 0:2])
                    xn = wsp.tile([P, P], F32, tag="xn", name="xn")
                    nc.scalar.activation(out=xn[:], in_=h_a[:, w, :],
                                         func=AF.Identity, scale=mws[:, 1:2],
                                         bias=mws[:, 0:1])
                    nc.vector.tensor_tensor(out=xn[:], in0=xn[:], in1=lnw,
                                            op=OP.mult)
                    nc.vector.tensor_tensor(out=xn[:], in0=xn[:], in1=lnb,
                                            op=OP.add)
                    # elu = max(x,0) + exp(min(x,0)) - 1
                    mn = wsp.tile([P, P], F32, tag="mn", name="mn")
                    nc.vector.tensor_scalar(out=mn[:], in0=xn[:], scalar1=0.0,
                                            scalar2=None, op0=OP.min)
                    nc.scalar.activation(out=mn[:], in_=mn[:], func=AF.Exp)
                    mx = wsp.tile([P, P], F32, tag="mx", name="mx")
                    nc.vector.tensor_scalar(out=mx[:], in0=xn[:], scalar1=0.0,
                                            scalar2=None, op0=OP.max)
                    nc.vector.tensor_tensor(out=mx[:], in0=mx[:], in1=mn[:],
                                            op=OP.add)
                    if add_resid:
                        nc.vector.tensor_scalar(out=mx[:], in0=mx[:],
                                                scalar1=1.0, scalar2=None,
                                                op0=OP.subtract)
                        rt = wsp.tile([P, P], BF, tag="rt", name="rt")
                        nc.sync.dma_start(out=rt[:],
                                          in_=rs_d[:, w * P:(w + 1) * P])
                        nc.vector.tensor_tensor(out=h_a[:, w, :], in0=mx[:],
                                                in1=rt[:], op=OP.add)
                    else:
                        nc.vector.tensor_scalar(out=h_a[:, w, :], in0=mx[:],
                                                scalar1=1.0, scalar2=None,
                                                op0=OP.subtract)
                    stage_n(w, ps)
            return st_n.get("xr_all")

        # ------------------------------------------------------------------
        def final_layer():
            wlf = wpk_s[:, 6 * P:6 * P + DF]
            wrf = wpk_s[:, 6 * P + DF:6 * P + 2 * DF]

            with tc.tile_pool(name="psf", bufs=1, space="PSUM") as ps:
                for g in groups:
                    gq, boff = grp_gathers(g)
                    for w in g["ws"]:
                        base = int(woff[w])
                        Tww = int(meta["Tw"][w])
                        xrfp = ps.tile([P, DF], F32, tag="ep", bufs=3,
                                       name="xrfp")
                        nc.tensor.matmul(out=xrfp[:], lhsT=hT[:, w, :],
                                         rhs=wrf, start=True, stop=True)
                        xrf = wsp.tile([P, DF], BF, tag="xrf", name="xrf")
                        nc.scalar.activation(out=xrf[:], in_=xrfp[:],
                                             func=AF.Identity)
                        ST = build_st(w, base, Tww)
                        ews2 = wsp.tile([2, TWMAX * P], BF, tag="ews2",
                                        name="ewsf")
                        nc.sync.dma_start(
                            out=ews2[0:2, :Tww * P],
                            in_=ew2_d[0:2, base * P:(base + Tww) * P])
                        cht = ps.tile([P, HEADS, P], F32, tag="cht",
                                      name="cht")
                        dnm = ps.tile([P, HEADS], F32, tag="fdnm", name="fdnm")
                        pend = []

                        def emit_cht(t0p, J, col0, lt0p, Bp, wqp, first,
                                     last, STx):
                            for j in range(J):
                                nc.tensor.matmul(
                                    out=cht[:].rearrange("p h c -> p (h c)"),
                                    lhsT=gq[:, col0 + j, :],
                                    rhs=Bp[:, j, :, :].rearrange(
                                        "p h c -> p (h c)"),
                                    start=(first and j == 0),
                                    stop=(last and j == J - 1))
                                nc.tensor.matmul(out=dnm[:],
                                                 lhsT=STx[:, :, lt0p + j],
                                                 rhs=wqp[:, j, :],
                                                 start=(first and j == 0),
                                                 stop=(last and j == J - 1))

                        lastsplit = max(si for si in range(S)
                                        if TS[si, w] > 0)
                        first = True
                        for si in range(S):
                            Th = int(TS[si, w])
                            if Th == 0:
                                continue
                            gq0 = g["smap"][si][w] + int(boff[si])
                            lt0 = int(run_off[si, w])
                            for t0p in range(0, Th, 2):
                                J = min(2, Th - t0p)
                                ea2 = wsp.tile([P, 2, DF], BF, tag="fea",
                                               bufs=3, name="fea")
                                for j in range(J):
                                    lt = lt0 + t0p + j
                                    col = gq0 + t0p + j
                                    gp = ps.tile([P, 2, P], BF, tag="pt",
                                                 bufs=2, name="gp")
                                    nc.tensor.transpose(out=gp[:, 0, :],
                                                        in_=gq[:, col, :],
                                                        identity=identb[:])
                                    nc.tensor.transpose(out=gp[:, 1, :],
                                                        in_=ST[:, :, lt],
                                                        identity=identb[:])
                                    gS = wsp.tile([P, 2, P], BF, tag="ghT",
                                                  bufs=3, name="ghT")
                                    nc.scalar.activation(out=gS[:],
                                                         in_=gp[:],
                                                         func=AF.Identity)
                                    ep = ps.tile([P, DF], F32, tag="ep",
                                                 bufs=3, name="fep")
                                    nc.tensor.matmul(
                                        out=ep[:],
                                        lhsT=ews2[0:2, lt * P:(lt + 1) * P],
                                        rhs=fwebb_s[0:2, :],
                                        start=True, stop=False)
                                    nc.tensor.matmul(out=ep[:],
                                                     lhsT=gS[:, 0, :],
                                                     rhs=wlf, start=False,
                                                     stop=False)
                                    nc.tensor.matmul(out=ep[:],
                                                     lhsT=gS[:, 1, :],
                                                     rhs=xrf[:], start=False,
                                                     stop=True)
                                    leaky(ea2[:, j, :], ep[:], [P, DF])
                                lg2 = wsp.tile([P, 2, DF], BF, tag="flg",
                                               bufs=3, name="flg")
                                nc.vector.tensor_tensor(
                                    out=lg2[:, :J, :], in0=ea2[:, :J, :],
                                    in1=attf_s[:, None, :].to_broadcast(
                                        [P, J, DF]),
                                    op=OP.mult)
                                lgr2 = wsp.tile([P, 2 * HEADS], BF,
                                                tag="flgr", name="flgr")
                                with nc.allow_low_precision(
                                        reason="bf16 head-logit reduce"):
                                    nc.vector.tensor_reduce(
                                        out=lgr2[:, :J * HEADS].rearrange(
                                            "p (j h) -> p j h", h=HEADS),
                                        in_=lg2[:, :J, :].rearrange(
                                            "p j (h c) -> p j h c", h=HEADS,
                                            c=P),
                                        axis=AX.X, op=OP.add)
                                wqp = wsp.tile([P, 2, HEADS], BF, tag="fwq",
                                               bufs=3, name="fwq")
                                nc.scalar.activation(
                                    out=wqp[:, :J, :],
                                    in_=lgr2[:, :J * HEADS].rearrange(
                                        "p (j h) -> p j h", h=HEADS),
                                    func=AF.Exp)
                                Bp = wsp.tile([P, 2, HEADS, P], BF, tag="fB",
                                              bufs=3, name="fB")
                                nc.vector.tensor_tensor(
                                    out=Bp[:, :J, :, :],
                                    in0=ST[:, :, lt0 + t0p:lt0 + t0p + J]
                                        .rearrange("p d j -> p j d")
                                        [:, :, None, :]
                                        .to_broadcast([P, J, HEADS, P]),
                                    in1=wqp[:, :J, :, None].to_broadcast(
                                        [P, J, HEADS, P]),
                                    op=OP.mult)
                                last = (si == lastsplit and
                                        t0p + J == Th)
                                if pend:
                                    emit_cht(*pend.pop())
                                pend.append((t0p, J, gq0 + t0p, lt0 + t0p,
                                             Bp, wqp, first, last, ST))
                                first = False
                        for args in pend:
                            emit_cht(*args)

                        # flush: nmr_h = ChT_h^T @ Wlf_h; out = bias +
                        #        mean_h numer/denom
                        chsb = wsp.tile([P, HEADS, P], BF, tag="chsb",
                                        name="chsb")
                        nc.scalar.activation(out=chsb[:], in_=cht[:],
                                             func=AF.Identity)
                        nmr = ps.tile([P, DF], F32, tag="ep", bufs=3,
                                      name="fnmr")
                        for h in range(HEADS):
                            nc.tensor.matmul(
                                out=nmr[:, h * P:(h + 1) * P],
                                lhsT=chsb[:, h, :],
                                rhs=wlf[:, h * P:(h + 1) * P],
                                start=True, stop=True)
                        rd = wsp.tile([P, HEADS], F32, tag="rd", name="frd")
                        nc.vector.tensor_scalar(out=rd[:], in0=dnm[:],
                                                scalar1=1e-16, scalar2=None,
                                                op0=OP.add)
                        nc.vector.reciprocal(out=rd[:], in_=rd[:])
                        nc.vector.tensor_scalar(out=rd[:], in0=rd[:],
                                                scalar1=1.0 / HEADS,
                                                scalar2=None, op0=OP.mult)
                        sc = wsp.tile([P, HEADS, P], F32, tag="sc", bufs=1,
                                      name="sc")
                        nc.vector.tensor_tensor(
                            out=sc[:],
                            in0=nmr[:].rearrange("p (h c) -> p h c", h=HEADS,
                                                 c=P),
                            in1=rd[:, :, None].to_broadcast([P, HEADS, P]),
                            op=OP.mult)
                        acc = wsp.tile([P, P], F32, tag="acc", name="acc")
                        nc.vector.tensor_reduce(
                            out=acc[:], in_=sc[:].rearrange("p h c -> p c h"),
                            axis=AX.X, op=OP.add)
                        nc.vector.tensor_tensor(out=acc[:], in0=acc[:],
                                                in1=fpk_s[:], op=OP.add)
                        nc.sync.dma_start(out=out_d[w * P:(w + 1) * P, :],
                                          in_=acc[:])

        # ---- the 4 layers (layer li+1's staging fused into li's P3) ----
        with tc.tile_pool(name="psS", bufs=1, space="PSUM") as psS:
            st0, stage0 = make_stager(0)
            for w in range(NW):
                stage0(w, psS)
        xr = st0["xr_all"]
        xr = hidden_layer(0, False, xr, 1)
        xr = hidden_layer(1, True, xr, 2)
        hidden_layer(2, False, xr, 3)
        final_layer()

    nc.compile()
    return nc


# ----------------------------------------------------------------------------
# Host-side driver
# ----------------------------------------------------------------------------

def _repP(v):
    v = np.asarray(v, np.float32).reshape(-1)
    return np.broadcast_to(v, (P, v.shape[0]))


def _winmaj(arr, lo_c, hi_c, NW):
    """[n, P] node-major slice -> [P, NW*P] window-major (padded)."""
    out = np.zeros((NW * P, P), np.float32)
    out[:hi_c - lo_c] = arr[lo_c:hi_c]
    return np.ascontiguousarray(
        out.reshape(NW, P, P).transpose(1, 0, 2).reshape(P, NW * P))


def make_in_maps(meta, inputs):
    NW, TT = meta["NW"], meta["TT"]
    lo, hi = meta["lo"], meta["hi"]
    x = np.asarray(inputs["x"], np.float32)
    resid = np.asarray(inputs["residual"], np.float32)
    ew = np.asarray(inputs["edge_weight"], np.float32)

    att = np.asarray(inputs["att"], np.float32)      # (3, H, C)
    attf = np.asarray(inputs["att_f"], np.float32)   # (H, DOUT)
    bl = np.asarray(inputs["bl"], np.float32)
    br = np.asarray(inputs["br"], np.float32)
    bias = np.asarray(inputs["bias"], np.float32)
    blf = np.asarray(inputs["bl_f"], np.float32)
    brf = np.asarray(inputs["br_f"], np.float32)
    biasf = np.asarray(inputs["bias_f"], np.float32)

    bf16 = ml_dtypes.bfloat16
    # hidden features stored (c h)-interleaved so the DVE alpha-weighting
    # multiply has a packed last dim (2x mode); PRM[c*H+h] = h*CH+c
    PRM = np.array([h * CH + c for c in range(CH) for h in range(HEADS)])
    wpk = np.concatenate(
        [np.asarray(inputs["Wl"], np.float32)[i][:, PRM] for i in range(3)]
        + [np.asarray(inputs["Wr"], np.float32)[i][:, PRM] for i in range(3)]
        + [np.asarray(inputs["Wl_f"], np.float32),
           np.asarray(inputs["Wr_f"], np.float32)], axis=1).astype(bf16)
    cpk = np.concatenate(
        [_repP(inputs["ln_w"][i]) for i in range(3)]
        + [_repP(inputs["ln_b"][i]) for i in range(3)]
        + [_repP(bias[i] + bl[i]) for i in range(3)], axis=1).astype(np.float32)
    apk = np.concatenate([_repP(att[i].reshape(-1)[PRM]) for i in range(3)],
                         axis=1).astype(bf16)
    We = np.asarray(inputs["We"], np.float32)
    webb = np.zeros((5, 12 * P), np.float32)
    for l in range(3):
        webb[0, l * 4 * P:(l + 1) * 4 * P] = np.tile((bl[l] + br[l])[PRM], 4)
        for r in range(4):
            webb[1 + r, l * 4 * P + r * P:l * 4 * P + (r + 1) * P] = \
                We[l].reshape(P)[PRM]
    fwebb = np.stack([blf + brf,
                      np.asarray(inputs["We_f"], np.float32).reshape(DF)])
    attf_rep = _repP(attf).astype(bf16)
    biaf_eff = biasf + blf.reshape(HEADS, -1).mean(axis=0)
    fpk = _repP(biaf_eff).astype(np.float32)

    common = dict(invd=meta["invd"].astype(np.float32), cpk=cpk, apk=apk,
                  wpk=wpk, attf=attf_rep, fpk=fpk,
                  webb=webb.astype(bf16), fwebb=fwebb.astype(bf16))

    S = meta["S"]
    woff, TS, run_off = meta["woff"], meta["TS"], meta["run_off"]
    nqcum, qoff, NQTOT = meta["nqcum"], meta["qoff"], meta["NQTOT"]

    in_maps = []
    for c in range(NCORES):
        ewc = np.zeros(TT * P, np.float32)
        m = meta["ewsl"][c] >= 0
        ewc[m] = ew[meta["ewsl"][c][m]]
        ew2 = np.stack([np.ones(TT * P, np.float32), ewc])
        ew5 = np.zeros((5, NQTOT * P), np.float32)
        ew5[0] = 1.0
        for w in range(NW):
            for si in range(S):
                Th = int(TS[si, w])
                lt0 = int(run_off[si, w])
                qq0 = int(qoff[w]) + int(nqcum[si, w])
                for q0 in range(0, Th, 4):
                    qq = qq0 + q0 // 4
                    for r in range(min(4, Th - q0)):
                        gt = int(woff[w]) + lt0 + q0 + r
                        ew5[1 + r, qq * P:(qq + 1) * P] = \
                            ewc[gt * P:(gt + 1) * P]
        in_maps.append(dict(
            h0s=_winmaj(x, lo[c], hi[c], NW).astype(bf16),
            rs=_winmaj(resid, lo[c], hi[c], NW).astype(bf16),
            dsti=meta["dsti"][c].astype(bf16),
            idx=np.tile(meta["idxm"][c], (8, 1)),
            ew5=ew5.astype(bf16),
            ew2=ew2.astype(bf16),
            gmat=meta["gmat"][c].astype(bf16),
            gmatT=meta["gmatT"][c].astype(bf16),
            **common))
    return in_maps


def assemble(meta, results):
    N = meta["N"]
    lo, hi = meta["lo"], meta["hi"]
    out = np.zeros((N, P), np.float32)
    for c in range(NCORES):
        n = int(hi[c] - lo[c])
        out[lo[c]:hi[c]] = results[c]["out"][:n]
    return out


_CACHE = {}


def kernel(**inputs):
    ei = np.asarray(inputs["edge_index"])
    bt = np.asarray(inputs["batch"])
    key = (ei.shape, bt.shape, hash(ei.tobytes()), hash(bt.tobytes()))
    if key not in _CACHE:
        meta = build_meta(ei, bt)
        nc = build_program(meta)
        _CACHE[key] = (meta, nc)
    meta, nc = _CACHE[key]
    in_maps = make_in_maps(meta, inputs)
    res = run_bass_kernel_spmd(nc, in_maps, list(range(NCORES)))
    return assemble(meta, res.results)



# revision 31
# speedup vs baseline: 1.0479x; 1.0003x over previous
"""GATv2 backbone (4 layers) on 8 Trainium2 NeuronCores.

Strategy (v4):
  * v3 + cross-layer phase fusion: each layer's normalize (P3) stages the
    NEXT layer's tables per window (transpose + xl/xr projection + DRAM
    bounce) and fires the per-band AllGathers early, removing the serial
    P3->P0 boundary. Gathers issue with single_packet=False (descriptors
    spread over all 16 SDMA engines); deeper tile buffering (gq bufs=3,
    per-quad chain bufs=4) keeps more quads in flight across the
    10-step cross-engine dependency chain.

Strategy (v3):
  * Nodes partitioned into 8 contiguous ranges (edge-balanced); edges owned
    by the dst core, grouped by 128-node dst windows. Windows are divided
    into SPLITS contiguous bands; each band's node table (<=32767 rows, so
    int16 gather indices work) is AllGathered separately, and the AllGathers
    are issued as soon as their band's xl shard is staged -- they pipeline
    with P0 and with edge-phase compute of earlier bands.
  * Gathers: nc.gpsimd.dma_gather, <=8 tiles (1024 rows) per op, one op per
    (window-group, band, chunk) -- ~130 ops/layer vs 850 indirect DMAs.
  * Hidden layers gather xl rows (256B); the final layer gathers h rows
    (256B) and computes xlf = h @ Wlf on-chip (4x less gather + AllGather
    traffic), with the numerator factored as (B_h^T @ gh) @ Wlf_h.
  * One-hot ST built bf16 in [e, d, t] layout (packed last dim -> DVE 2x
    mode); S = PE transpose of ST slices, PSUM->SBUF copies on the ACT
    engine. Edge-weight rank-1s and both linear biases fold into a single
    [ones; ew-rows] @ [bias; We-blocks] matmul per quad; numerator bias
    recovered via sum(alpha)=1. nmr|dnm share one PSUM accumulation matmul.
  * leaky_relu via Prelu (same ACT table set as Exp/Square -> no reloads).
"""

import contextlib
import os

import ml_dtypes
import numpy as np

from concourse import bass, bacc, mybir, tile
from concourse.bass_utils import run_bass_kernel_spmd
from concourse.masks import make_identity

P = 128
NCORES = 8
GMAX = 50
HEADS = 4
DHID = 128
CH = DHID // HEADS          # 32
DF = 512
NEG = 0.2
EPS = 1e-5
GW = 1                      # windows per gather group
MAXT = 8                    # tiles per dma_gather op (1024 idxs, HW limit)
NSPLIT = 4                  # table bands (pipelined AllGathers)
NQUEUES = 4                 # SWDGE queues for gather DMAs
USE_LRELU = os.environ.get("K_NO_PRELU", "") != "1"

F32 = mybir.dt.float32
BF = mybir.dt.bfloat16
I32 = mybir.dt.int32
I16 = mybir.dt.int16
AX = mybir.AxisListType
OP = mybir.AluOpType
AF = mybir.ActivationFunctionType


# ----------------------------------------------------------------------------
# Host preprocessing: graph partitioning + static schedule
# ----------------------------------------------------------------------------

def build_meta(edge_index, batch):
    N = batch.shape[0]
    E = edge_index.shape[1]
    src = np.asarray(edge_index[0], dtype=np.int64)
    dst = np.asarray(edge_index[1], dtype=np.int64)
    batch = np.asarray(batch, dtype=np.int64)

    deg = np.bincount(dst, minlength=N)
    cum = np.concatenate([[0], np.cumsum(deg)])

    bounds = [0]
    for c in range(1, NCORES):
        n = int(np.searchsorted(cum, c * E / NCORES))
        bounds.append(min(max(n, bounds[-1] + 1), N - (NCORES - c)))
    bounds.append(N)
    lo = np.array(bounds[:-1])
    hi = np.array(bounds[1:])

    NW = int(max((hi - lo + P - 1) // P))
    NPAD = NW * P
    S = min(NSPLIT, NW)
    swin = [a for a in np.array_split(np.arange(NW), S)]
    sbase = np.array([int(a[0]) for a in swin])
    scount = np.array([len(a) for a in swin])
    TBLROWS = NCORES * scount * P
    assert (TBLROWS <= 32767).all(), TBLROWS
    split_id = np.zeros(NW, np.int64)
    for si, a in enumerate(swin):
        split_id[a] = si

    core_of = np.repeat(np.arange(NCORES), (hi - lo))
    off = np.arange(N) - lo[core_of]
    wof = off // P                       # window-within-core (may pad-overflow
    wof = np.minimum(wof, NW - 1)        # never: off < NPAD)
    nsp = split_id[wof]
    nrow = core_of * scount[nsp] * P + (off - sbase[nsp] * P)

    order = np.argsort(dst, kind="stable")

    cnt = np.zeros((S, NCORES, NW), np.int64)
    lists = {}
    for c in range(NCORES):
        for w in range(NW):
            a = lo[c] + w * P
            b = min(a + P, hi[c])
            if a >= b:
                lists[(c, w)] = [np.empty(0, np.int64)] * S
                continue
            ids = order[int(cum[a]):int(cum[b])]
            sp = nsp[src[ids]]
            per = []
            for si in range(S):
                idsS = ids[sp == si]
                idsS = idsS[np.argsort(nrow[src[idsS]], kind="stable")]
                per.append(idsS)
                cnt[si, c, w] = len(idsS)
            lists[(c, w)] = per

    TS = ((cnt.max(axis=1) + P - 1) // P).astype(np.int64)    # [S, NW]
    for w in range(NW):
        if TS[:, w].sum() == 0:
            TS[0, w] = 1
    run_off = np.zeros((S, NW), np.int64)
    for si in range(1, S):
        run_off[si] = run_off[si - 1] + TS[si - 1]
    Tw = TS.sum(axis=0)
    woff = np.concatenate([[0], np.cumsum(Tw)]).astype(np.int64)
    TT = int(woff[-1])

    dsti = np.full((NCORES, P, TT), -1, np.int64)
    ewsl = np.full((NCORES, TT * P), -1, np.int64)
    srow = np.zeros((NCORES, TT * P), np.int64)
    for c in range(NCORES):
        for w in range(NW):
            for si in range(S):
                ids_h = lists[(c, w)][si]
                n = len(ids_h)
                if n == 0:
                    continue
                tb = int(woff[w] + run_off[si, w])
                slot = np.arange(n)
                tt = tb + slot // P
                pp = slot % P
                dsti[c, pp, tt] = dst[ids_h] - (lo[c] + w * P)
                flat = tt * P + pp
                ewsl[c, flat] = ids_h
                srow[c, flat] = nrow[src[ids_h]]

    # per-window quad schedule (band-major)
    nqS = ((TS + 3) // 4).astype(np.int64)            # [S, NW]
    nqcum = np.zeros((S, NW), np.int64)
    for si in range(1, S):
        nqcum[si] = nqcum[si - 1] + nqS[si - 1]
    nquad = nqS.sum(axis=0)
    NQBMAX = int(nqS.max())
    qoff = np.concatenate([[0], np.cumsum(nquad)]).astype(np.int64)
    NQTOT = int(qoff[-1])
    NQMAX = int(nquad.max())

    # gather-op schedule (static, identical across cores)
    groups = []
    icols = 0
    for g0 in range(0, NW, GW):
        ws = list(range(g0, min(g0 + GW, NW)))
        smap = [dict() for _ in range(S)]
        tgb = []
        ops = []
        for si in range(S):
            run = 0
            for w in ws:
                smap[si][w] = run
                run += int(TS[si, w])
            tgb.append(run)
            tiles = []
            for w in ws:
                for t in range(int(TS[si, w])):
                    gt = int(woff[w] + run_off[si, w]) + t
                    tiles.append((gt, smap[si][w] + t))
            for i in range(0, len(tiles), MAXT):
                ch = tiles[i:i + MAXT]
                ops.append(dict(split=si, coff=icols, nt=len(ch),
                                gq0=ch[0][1], gtiles=[x[0] for x in ch]))
                icols += len(ch) * 8
        groups.append(dict(ws=ws, tgb=tgb, smap=smap, ops=ops))
    ICOLS = icols
    TGMAX = max(max(g["tgb"]) for g in groups)
    TGFULL = max(sum(g["tgb"]) for g in groups)
    TWMAX = int(Tw.max())
    fnz = np.array([min(si for si in range(S) if TS[si, w] > 0)
                    for w in range(NW)])
    lnz = np.array([max(si for si in range(S) if TS[si, w] > 0)
                    for w in range(NW)])

    idxm = np.zeros((NCORES, 16, ICOLS), np.int16)
    for c in range(NCORES):
        for g in groups:
            for op in g["ops"]:
                nt = op["nt"]
                vals = np.zeros(nt * P, np.int64)
                for j, gt in enumerate(op["gtiles"]):
                    vals[j * P:(j + 1) * P] = srow[c, gt * P:(gt + 1) * P]
                idxm[c, :, op["coff"]:op["coff"] + nt * 8] = (
                    vals.reshape(nt * 8, 16).T.astype(np.int16))

    gmat = np.zeros((NCORES, P, NW * GMAX), np.float32)
    gmatT = np.zeros((NCORES, GMAX, NW * P), np.float32)
    for c in range(NCORES):
        nreal = int(hi[c] - lo[c])
        g = batch[lo[c]:hi[c]]
        r = np.arange(nreal)
        gmat[c, r % P, (r // P) * GMAX + g] = 1.0
        gmatT[c, g, (r // P) * P + (r % P)] = 1.0

    cntg = np.bincount(batch, minlength=GMAX).astype(np.float32)
    invd = (1.0 / (np.maximum(cntg, 1.0) * DHID)).reshape(1, GMAX)

    return dict(N=N, E=E, NW=NW, NPAD=NPAD, S=S, swin=swin, sbase=sbase,
                scount=scount, TBLROWS=TBLROWS, TT=TT, ICOLS=ICOLS,
                TGMAX=TGMAX, TWMAX=TWMAX, TS=TS, run_off=run_off, Tw=Tw,
                woff=woff, lo=lo, hi=hi, nqS=nqS, nqcum=nqcum, nquad=nquad,
                qoff=qoff, NQTOT=NQTOT, NQMAX=NQMAX, NQBMAX=NQBMAX,
                TGFULL=TGFULL,
                groups=groups, fnz=fnz, lnz=lnz,
                dsti=dsti, ewsl=ewsl, idxm=idxm, gmat=gmat, gmatT=gmatT,
                invd=invd)


# ----------------------------------------------------------------------------
# Bass program
# ----------------------------------------------------------------------------

def build_program(meta):
    NW, NPAD, TT = meta["NW"], meta["NPAD"], meta["TT"]
    S, sbase, scount = meta["S"], meta["sbase"], meta["scount"]
    TBLROWS, ICOLS = meta["TBLROWS"], meta["ICOLS"]
    TGMAX, TWMAX = meta["TGMAX"], meta["TWMAX"]
    TS, run_off, woff = meta["TS"], meta["run_off"], meta["woff"]
    nqS, nqcum, nquad, qoff = (meta["nqS"], meta["nqcum"], meta["nquad"],
                               meta["qoff"])
    NQTOT, NQMAX = meta["NQTOT"], meta["NQMAX"]
    NQBMAX = meta["NQBMAX"]
    TGFULL = meta["TGFULL"]
    groups = meta["groups"]
    swin = meta["swin"]
    fnz, lnz = meta["fnz"], meta["lnz"]

    nc = bacc.Bacc("TRN2", target_bir_lowering=False, debug=False,
                   enable_asserts=False, num_devices=NCORES,
                   num_swdge_queues=1)

    h0s_d = nc.dram_tensor("h0s", [P, NW * P], BF, kind="ExternalInput")
    rs_d = nc.dram_tensor("rs", [P, NW * P], BF, kind="ExternalInput")
    dsti_d = nc.dram_tensor("dsti", [P, TT], BF, kind="ExternalInput")
    idx_d = nc.dram_tensor("idx", [P, ICOLS], I16, kind="ExternalInput")
    ew5_d = nc.dram_tensor("ew5", [5, NQTOT * P], BF, kind="ExternalInput")
    ew2_d = nc.dram_tensor("ew2", [2, TT * P], BF, kind="ExternalInput")
    gmat_d = nc.dram_tensor("gmat", [P, NW * GMAX], BF, kind="ExternalInput")
    gmatT_d = nc.dram_tensor("gmatT", [GMAX, NW * P], BF, kind="ExternalInput")
    invd_d = nc.dram_tensor("invd", [1, GMAX], F32, kind="ExternalInput")
    cpk_d = nc.dram_tensor("cpk", [P, 9 * P], F32, kind="ExternalInput")
    apk_d = nc.dram_tensor("apk", [P, 3 * P], BF, kind="ExternalInput")
    wpk_d = nc.dram_tensor("wpk", [P, 6 * P + 2 * DF], BF, kind="ExternalInput")
    attf_d = nc.dram_tensor("attf", [P, DF], BF, kind="ExternalInput")
    fpk_d = nc.dram_tensor("fpk", [P, P], F32, kind="ExternalInput")
    webb_d = nc.dram_tensor("webb", [5, 12 * P], BF, kind="ExternalInput")
    fwebb_d = nc.dram_tensor("fwebb", [2, DF], BF, kind="ExternalInput")
    out_d = nc.dram_tensor("out", [NPAD, P], F32, kind="ExternalOutput")

    with tile.TileContext(nc) as tc, contextlib.ExitStack() as ctx:
        dram = ctx.enter_context(tc.tile_pool(name="dram", bufs=1, space="DRAM"))
        cst = ctx.enter_context(tc.tile_pool(name="cst", bufs=1))
        per = ctx.enter_context(tc.tile_pool(name="per", bufs=1))
        wsp = ctx.enter_context(tc.tile_pool(name="wsp", bufs=2))
        gpo = ctx.enter_context(tc.tile_pool(name="gpo", bufs=2))

        xl_b = dram.tile([NPAD, P], BF)
        tbl = [nc.dram_tensor(f"tbl{si}", [int(TBLROWS[si]), P], BF,
                              kind="Internal", addr_space="Shared")
               for si in range(S)]
        st_b = dram.tile([2, GMAX], F32)
        st_o = dram.tile([2, GMAX], F32)
        cgroups = [list(range(NCORES))]

        # --- constants / residents ---
        ident = cst.tile([P, P], F32)
        make_identity(nc, ident[:])
        identb = cst.tile([P, P], BF)
        nc.vector.tensor_copy(out=identb[:], in_=ident[:])
        iota_row = cst.tile([P, P], I32)
        nc.gpsimd.iota(iota_row[:], pattern=[[1, P]], base=0,
                       channel_multiplier=0)
        iota_rowb = cst.tile([P, P], BF)
        nc.vector.tensor_copy(out=iota_rowb[:], in_=iota_row[:])
        iota3 = cst.tile([P, P, TWMAX], BF)
        nc.vector.tensor_copy(
            out=iota3[:],
            in_=iota_rowb[:, :, None].to_broadcast([P, P, TWMAX]))
        epsc = cst.tile([P, 1], F32)
        nc.vector.memset(epsc[:], EPS)
        invd = cst.tile([1, GMAX], F32)
        nc.sync.dma_start(out=invd[:], in_=invd_d[:, :])
        wpk_s = cst.tile([P, 6 * P + 2 * DF], BF)
        nc.sync.dma_start(out=wpk_s[:], in_=wpk_d[:, :])
        dsti_s = cst.tile([P, TT], BF)
        nc.sync.dma_start(out=dsti_s[:], in_=dsti_d[:, :])
        idx_s = cst.tile([P, ICOLS], I16)
        nc.sync.dma_start(out=idx_s[:], in_=idx_d[:, :])
        gmat_s = cst.tile([P, NW * GMAX], BF)
        nc.sync.dma_start(out=gmat_s[:], in_=gmat_d[:, :])
        gmatT_s = cst.tile([GMAX, NW * P], BF)
        nc.sync.dma_start(out=gmatT_s[:], in_=gmatT_d[:, :])
        cpk_s = cst.tile([P, 9 * P], F32)
        nc.sync.dma_start(out=cpk_s[:], in_=cpk_d[:, :])
        apk_s = cst.tile([P, 3 * P], BF)
        nc.sync.dma_start(out=apk_s[:], in_=apk_d[:, :])
        attf_s = cst.tile([P, DF], BF)
        nc.sync.dma_start(out=attf_s[:], in_=attf_d[:, :])
        fpk_s = cst.tile([P, P], F32)
        nc.sync.dma_start(out=fpk_s[:], in_=fpk_d[:, :])
        webb_s = cst.tile([5, 12 * P], BF)
        nc.sync.dma_start(out=webb_s[:], in_=webb_d[:, :])
        fwebb_s = cst.tile([2, DF], BF)
        nc.sync.dma_start(out=fwebb_s[:], in_=fwebb_d[:, :])

        h_a = per.tile([P, NW, P], BF, tag="h_a")
        hT = per.tile([P, NW, P], BF, tag="hT")
        nc.sync.dma_start(out=h_a[:, :, :],
                          in_=h0s_d[:, :].rearrange("p (w f) -> p w f", w=NW))

        def leaky(dst_ap, src_ap, shape):
            if USE_LRELU:
                # Prelu == leaky relu with param alpha; unlike Lrelu it is in
                # the same ACT table set as Exp/Square -> no table reloads.
                nc.scalar.activation(out=dst_ap, in_=src_ap, func=AF.Prelu,
                                     alpha=NEG)
            else:
                r = wsp.tile(shape, F32, tag="lrtmp", bufs=1, name="lr")
                rr = r[tuple(slice(0, s) for s in dst_ap.shape)]
                nc.scalar.activation(out=rr, in_=src_ap, func=AF.Relu,
                                     scale=-(1.0 - NEG))
                nc.vector.tensor_tensor(out=dst_ap, in0=src_ap, in1=rr,
                                        op=OP.add)

        split_last = {int(a[-1]): si for si, a in enumerate(swin)}
        SGW = 2                 # staging chunk (windows per xl_b DMA)
        breaks = set(range(SGW - 1, NW, SGW)) | set(split_last) | {NW - 1}

        def emit_ag(si):
            a = int(sbase[si]) * P
            b = a + int(scount[si]) * P
            nc.gpsimd.collective_compute(
                "AllGather", OP.bypass, replica_groups=cgroups,
                ins=[xl_b[a:b, :].opt()], outs=[tbl[si][:, :].opt()])

        qctr = [0]

        def grp_gathers(g, band=None):
            if band is not None:
                gq = gpo.tile([P, TGMAX, P], BF, tag="gq", name="gq",
                              bufs=5)
            else:
                gq = gpo.tile([P, TGFULL, P], BF, tag="gqf", name="gqf",
                              bufs=3)
            boff = np.concatenate([[0], np.cumsum(g["tgb"])]).astype(int)
            for op in g["ops"]:
                if band is not None and op["split"] != band:
                    continue
                g0 = op["gq0"] + (0 if band is not None
                                  else int(boff[op["split"]]))
                nc.gpsimd.dma_gather(
                    gq[:, g0:g0 + op["nt"], :],
                    tbl[op["split"]][:, :],
                    idx_s[:, op["coff"]:op["coff"] + op["nt"] * 8],
                    op["nt"] * P, op["nt"] * P, P,
                    single_packet=False)
            return gq, boff

        def build_st(w, t0, Tn):
            """ST2[e, d, t] one-hot (bf16, packed last dim -> DVE 2x)."""
            ST = wsp.tile([P, P, TWMAX], BF, tag="ST", name="ST")
            nc.vector.tensor_tensor(
                out=ST[:, :, :Tn],
                in0=iota3[:, :, :Tn],
                in1=dsti_s[:, None, t0:t0 + Tn]
                    .to_broadcast([P, P, Tn]),
                op=OP.is_equal)
            return ST

        # ------------------------------------------------------------------
        def make_stager(li):
            """Staging for layer li's tables: transpose h, project (hidden)
            or copy (final), bounce to DRAM, fire per-band AllGathers.
            Called per window, fused into the previous layer's P3."""
            st = dict(run=False, w0=0, xsg=None)
            if li < 3:
                st["xr_all"] = wsp.tile([P, NW, P], BF, tag="xra", bufs=1,
                                        name="xra")
                wl = wpk_s[:, li * P:(li + 1) * P]
                wr = wpk_s[:, (3 + li) * P:(4 + li) * P]

            def stage(w, ps):
                nc.sync.dma_start(out=hT[:, w, :], in_=h_a[:, w, :],
                                  transpose=True)
                if not st["run"]:
                    st["run"] = True
                    st["w0"] = w
                    if li < 3:
                        st["xsg"] = wsp.tile([P, SGW, P], BF, tag="xsg",
                                             name="xsg")
                w0 = st["w0"]
                if li < 3:
                    xp = ps.tile([P, P], F32, tag="px", bufs=2, name="px")
                    nc.tensor.matmul(out=xp[:], lhsT=hT[:, w, :], rhs=wl,
                                     start=True, stop=True)
                    nc.scalar.activation(out=st["xsg"][:, w - w0, :],
                                         in_=xp[:], func=AF.Identity)
                    xrp = ps.tile([P, P], F32, tag="px", bufs=2, name="xrp")
                    nc.tensor.matmul(out=xrp[:], lhsT=hT[:, w, :], rhs=wr,
                                     start=True, stop=True)
                    nc.scalar.activation(out=st["xr_all"][:, w, :],
                                         in_=xrp[:], func=AF.Identity)
                    if w in breaks:
                        nc.sync.dma_start(
                            out=xl_b[w0 * P:(w + 1) * P, :].rearrange(
                                "(w p) f -> p w f", p=P),
                            in_=st["xsg"][:, :w - w0 + 1, :])
                        st["run"] = False
                else:
                    if w in breaks:
                        nc.sync.dma_start(
                            out=xl_b[w0 * P:(w + 1) * P, :].rearrange(
                                "(w p) f -> p w f", p=P),
                            in_=h_a[:, w0:w + 1, :])
                        st["run"] = False
                if w in split_last:
                    emit_ag(split_last[w])
            return st, stage

        def hidden_layer(li, add_resid, xr_all, next_li):
            attr = apk_s[:, li * P:(li + 1) * P]
            lnw = cpk_s[:, li * P:(li + 1) * P]
            lnb = cpk_s[:, (3 + li) * P:(4 + li) * P]
            bia = cpk_s[:, (6 + li) * P:(7 + li) * P]

            with tc.tile_pool(name=f"ps{li}", bufs=1, space="PSUM") as ps:
                # P2: edge pipeline, band-major so AllGather si+1 overlaps
                # band-si compute; per-window numerators accumulate in SBUF.
                nd_all = wsp.tile([P, NW, P + HEADS], BF, tag="nda",
                                  bufs=1, name="nda")
                for band in range(S):
                    for g in groups:
                        if all(TS[band, w] == 0 for w in g["ws"]):
                            continue
                        gq, _ = grp_gathers(g, band)
                        gqv = gq[:].rearrange("p t (c h) -> p t c h",
                                              h=HEADS, c=CH)
                        for w in g["ws"]:
                            Th = int(TS[band, w])
                            if Th == 0:
                                continue
                            t0g = int(woff[w] + run_off[band, w])
                            ST = build_st(w, t0g, Th)
                            nqb = (Th + 3) // 4
                            qb0 = int(qoff[w] + nqcum[band, w])
                            ews5 = wsp.tile([5, NQBMAX * P], BF, tag="ews",
                                            bufs=4, name="ews")
                            nc.sync.dma_start(
                                out=ews5[0:5, :nqb * P],
                                in_=ew5_d[0:5, qb0 * P:(qb0 + nqb) * P])
                            nd = ps.tile([P, P + HEADS], F32, tag="nd",
                                         name="nd")
                            gq0 = g["smap"][band][w]

                            def emit_nd(q0, Q, mmw):
                                for t in range(Q):
                                    nc.tensor.matmul(
                                        out=nd[:], lhsT=ST[:, :, q0 + t],
                                        rhs=mmw[:, t, :],
                                        start=(q0 + t == 0),
                                        stop=(q0 + t == Th - 1))

                            pend = None
                            for q0 in range(0, Th, 4):
                                Q = min(4, Th - q0)
                                qq = q0 // 4
                                Ssb = wsp.tile([P, 4, P], BF, tag="ssb",
                                               bufs=5, name="ssb")
                                sp = ps.tile([P, 4, P], BF, tag="pt",
                                             bufs=2, name="sp")
                                for t in range(Q):
                                    nc.tensor.transpose(
                                        out=sp[:, t, :],
                                        in_=ST[:, :, q0 + t],
                                        identity=identb[:])
                                nc.scalar.activation(out=Ssb[:, :Q, :],
                                                     in_=sp[:, :Q, :],
                                                     func=AF.Identity)
                                ep = ps.tile([P, 4 * P], F32, tag="ep",
                                             bufs=2, name="ep")
                                # ef + biases first (host data, always ready)
                                # and the gather-dependent copy last, so the
                                # in-order PE queue head never parks on a
                                # not-yet-landed DMA.
                                nc.tensor.matmul(
                                    out=ep[:, :Q * P],
                                    lhsT=ews5[0:Q + 1, qq * P:(qq + 1) * P],
                                    rhs=webb_s[0:Q + 1,
                                               li * 4 * P:li * 4 * P + Q * P],
                                    start=True, stop=False)
                                for t in range(Q):
                                    blk = ep[:, t * P:(t + 1) * P]
                                    nc.tensor.matmul(out=blk,
                                                     lhsT=Ssb[:, t, :],
                                                     rhs=xr_all[:, w, :],
                                                     start=False, stop=False)
                                nc.tensor.matmul(
                                    out=ep[:, :Q * P], lhsT=identb[:],
                                    rhs=gq[:, gq0 + q0:gq0 + q0 + Q, :],
                                    start=False, stop=True)
                                ea = wsp.tile([P, 4 * P], BF, tag="ea", bufs=5,
                                              name="ea")
                                leaky(ea[:, :Q * P], ep[:, :Q * P], [P, 4 * P])
                                lg = wsp.tile([P, 4 * P], BF, tag="lg", bufs=5,
                                              name="lg")
                                nc.vector.tensor_tensor(
                                    out=lg[:, :Q * P], in0=ea[:, :Q * P],
                                    in1=attr[:, None, :].to_broadcast(
                                        [P, Q, P]),
                                    op=OP.mult)
                                lgr = wsp.tile([P, 4 * HEADS], F32, tag="lgr",
                                               bufs=4, name="lgr")
                                nc.vector.tensor_reduce(
                                    out=lgr[:, :Q * HEADS].rearrange(
                                        "p (t h) -> p t h", h=HEADS),
                                    in_=lg[:].rearrange(
                                        "p (t c h) -> p t h c", h=HEADS,
                                        c=CH)[:, :Q, :, :],
                                    axis=AX.X, op=OP.add)
                                mmw = wsp.tile([P, 4, P + HEADS], BF,
                                               tag="mm", bufs=4, name="mm")
                                nc.scalar.activation(
                                    out=mmw[:, :Q, P:P + HEADS],
                                    in_=lgr[:, :Q * HEADS].rearrange(
                                        "p (t h) -> p t h", h=HEADS),
                                    func=AF.Exp)
                                nc.vector.tensor_tensor(
                                    out=mmw[:, :Q, 0:P].rearrange(
                                        "p q (c h) -> p q c h", h=HEADS,
                                        c=CH),
                                    in0=gqv[:, gq0 + q0:gq0 + q0 + Q, :, :],
                                    in1=mmw[:, :Q, None, P:P + HEADS]
                                        .to_broadcast([P, Q, CH, HEADS]),
                                    op=OP.mult)
                                if pend is not None:
                                    emit_nd(*pend)
                                pend = (q0, Q, mmw)
                            if pend is not None:
                                emit_nd(*pend)
                            if band == int(fnz[w]):
                                nc.scalar.activation(out=nd_all[:, w, :],
                                                     in_=nd[:],
                                                     func=AF.Identity)
                            else:
                                nc.vector.tensor_tensor(
                                    out=nd_all[:, w, :], in0=nd_all[:, w, :],
                                    in1=nd[:], op=OP.add)

                # window flush + LN stats
                stp = ps.tile([2, GMAX], F32, tag="stats", name="stp")
                for w in range(NW):
                    rd = wsp.tile([P, HEADS], F32, tag="rd", name="rd")
                    nc.vector.tensor_scalar(out=rd[:],
                                            in0=nd_all[:, w, P:P + HEADS],
                                            scalar1=1e-16, scalar2=None,
                                            op0=OP.add)
                    nc.vector.reciprocal(out=rd[:], in_=rd[:])
                    oT = wsp.tile([P, HEADS, CH], F32, tag="oT", name="oT")
                    nc.vector.tensor_tensor(
                        out=oT[:],
                        in0=nd_all[:, w, :P].rearrange("p (c h) -> p h c",
                                                       h=HEADS, c=CH),
                        in1=rd[:, :, None].to_broadcast([P, HEADS, CH]),
                        op=OP.mult)
                    nc.vector.tensor_tensor(
                        out=h_a[:, w, :],
                        in0=oT[:].rearrange("p h c -> p (h c)"),
                        in1=bia, op=OP.add)
                    s12 = wsp.tile([P, 2], F32, tag="s12", name="s12")
                    nc.vector.tensor_reduce(out=s12[:, 0:1],
                                            in_=h_a[:, w, :],
                                            axis=AX.X, op=OP.add)
                    sqj = wsp.tile([P, P], BF, tag="sqj", name="sqj")
                    nc.scalar.activation(out=sqj[:], in_=h_a[:, w, :],
                                         func=AF.Square,
                                         accum_out=s12[:, 1:2])
                    s12b = wsp.tile([P, 2], BF, tag="s12b", name="s12b")
                    nc.vector.tensor_copy(out=s12b[:], in_=s12[:])
                    nc.tensor.matmul(
                        out=stp[:, :], lhsT=s12b[:],
                        rhs=gmat_s[:, w * GMAX:(w + 1) * GMAX],
                        start=(w == 0), stop=(w == NW - 1))

                # P3: stats -> mean/rstd -> normalize + elu
                sts = wsp.tile([2, GMAX], F32, tag="sts", name="sts")
                nc.vector.tensor_copy(out=sts[:], in_=stp[:])
                nc.sync.dma_start(out=st_b[:, :], in_=sts[:])
                nc.gpsimd.collective_compute(
                    "AllReduce", OP.add, replica_groups=cgroups,
                    ins=[st_b.opt()], outs=[st_o.opt()])
                stg1 = wsp.tile([1, GMAX], F32, tag="stg1", name="stg1")
                nc.sync.dma_start(out=stg1[:], in_=st_o[0:1, :])
                stg2 = wsp.tile([1, GMAX], F32, tag="stg2", name="stg2")
                nc.sync.dma_start(out=stg2[:], in_=st_o[1:2, :])
                mean = wsp.tile([1, GMAX], F32, tag="mean", name="mean")
                nc.vector.tensor_tensor(out=mean[:], in0=stg1[:],
                                        in1=invd[:], op=OP.mult)
                ex2 = wsp.tile([1, GMAX], F32, tag="ex2", name="ex2")
                nc.vector.tensor_tensor(out=ex2[:], in0=stg2[:],
                                        in1=invd[:], op=OP.mult)
                msq = wsp.tile([1, GMAX], F32, tag="msq", name="msq")
                nc.scalar.activation(out=msq[:], in_=mean[:], func=AF.Square)
                var = wsp.tile([1, GMAX], F32, tag="var", name="var")
                nc.vector.tensor_tensor(out=var[:], in0=ex2[:], in1=msq[:],
                                        op=OP.subtract)
                sd = wsp.tile([1, GMAX], F32, tag="sd", name="sd")
                nc.scalar.activation(out=sd[:], in_=var[:], func=AF.Sqrt,
                                     bias=epsc[0:1, 0:1])
                rstd = wsp.tile([1, GMAX], F32, tag="rstd", name="rstd")
                nc.vector.reciprocal(out=rstd[:], in_=sd[:])
                nmr2 = wsp.tile([1, GMAX], F32, tag="nmr2", name="nm2")
                nc.vector.tensor_tensor(out=nmr2[:], in0=mean[:], in1=rstd[:],
                                        op=OP.mult)
                nc.vector.tensor_scalar(out=nmr2[:], in0=nmr2[:], scalar1=-1.0,
                                        scalar2=None, op0=OP.mult)
                t1 = ps.tile([P, P], F32, tag="px", bufs=2, name="t1")
                nc.tensor.transpose(out=t1[0:GMAX, 0:1], in_=nmr2[:],
                                    identity=ident[0:1, 0:1])
                t2 = ps.tile([P, P], F32, tag="px", bufs=2, name="t2")
                nc.tensor.transpose(out=t2[0:GMAX, 0:1], in_=rstd[:],
                                    identity=ident[0:1, 0:1])
                nrcol = wsp.tile([GMAX, 2], BF, tag="nrcol", name="nrc")
                nc.vector.tensor_copy(out=nrcol[:, 0:1], in_=t1[0:GMAX, 0:1])
                nc.vector.tensor_copy(out=nrcol[:, 1:2], in_=t2[0:GMAX, 0:1])

                st_n, stage_n = make_stager(next_li)
                mwA = ps.tile([P, 2 * NW], F32, tag="stats", bufs=1,
                              name="mwA")
                for w in range(NW):
                    nc.tensor.matmul(out=mwA[:, 2 * w:2 * w + 2],
                                     lhsT=gmatT_s[:, w * P:(w + 1) * P],
                                     rhs=nrcol[:], start=True, stop=True)
                mws = wsp.tile([P, 2 * NW], F32, tag="mws", name="mws")
                nc.vector.tensor_copy(out=mws[:], in_=mwA[:])
                for w in range(NW):
                    xn = wsp.tile([P, P], F32, tag="xn", name="xn")
                    nc.scalar.activation(out=xn[:], in_=h_a[:, w, :],
                                         func=AF.Identity,
                                         scale=mws[:, 2 * w + 1:2 * w + 2],
                                         bias=mws[:, 2 * w:2 * w + 1])
                    nc.vector.tensor_tensor(out=xn[:], in0=xn[:], in1=lnw,
                                            op=OP.mult)
                    nc.vector.tensor_tensor(out=xn[:], in0=xn[:], in1=lnb,
                                            op=OP.add)
                    # elu = max(x,0) + exp(min(x,0)) - 1
                    mn = wsp.tile([P, P], F32, tag="mn", name="mn")
                    nc.vector.tensor_scalar(out=mn[:], in0=xn[:], scalar1=0.0,
                                            scalar2=None, op0=OP.min)
                    nc.scalar.activation(out=mn[:], in_=mn[:], func=AF.Exp)
                    mx = wsp.tile([P, P], F32, tag="mx", name="mx")
                    nc.vector.tensor_scalar(out=mx[:], in0=xn[:], scalar1=0.0,
                                            scalar2=None, op0=OP.max)
                    nc.vector.tensor_tensor(out=mx[:], in0=mx[:], in1=mn[:],
                                            op=OP.add)
                    if add_resid:
                        nc.vector.tensor_scalar(out=mx[:], in0=mx[:],
                                                scalar1=1.0, scalar2=None,
                                                op0=OP.subtract)
                        rt = wsp.tile([P, P], BF, tag="rt", name="rt")
                        nc.sync.dma_start(out=rt[:],
                                          in_=rs_d[:, w * P:(w + 1) * P])
                        nc.vector.tensor_tensor(out=h_a[:, w, :], in0=mx[:],
                                                in1=rt[:], op=OP.add)
                    else:
                        nc.vector.tensor_scalar(out=h_a[:, w, :], in0=mx[:],
                                                scalar1=1.0, scalar2=None,
                                                op0=OP.subtract)
                    stage_n(w, ps)
            return st_n.get("xr_all")

        # ------------------------------------------------------------------
        def final_layer():
            wlf = wpk_s[:, 6 * P:6 * P + DF]
            wrf = wpk_s[:, 6 * P + DF:6 * P + 2 * DF]

            with tc.tile_pool(name="psf", bufs=1, space="PSUM") as ps:
                for g in groups:
                    gq, boff = grp_gathers(g)
                    for w in g["ws"]:
                        base = int(woff[w])
                        Tww = int(meta["Tw"][w])
                        xrfp = ps.tile([P, DF], F32, tag="ep", bufs=3,
                                       name="xrfp")
                        nc.tensor.matmul(out=xrfp[:], lhsT=hT[:, w, :],
                                         rhs=wrf, start=True, stop=True)
                        xrf = wsp.tile([P, DF], BF, tag="xrf", name="xrf")
                        nc.scalar.activation(out=xrf[:], in_=xrfp[:],
                                             func=AF.Identity)
                        ST = build_st(w, base, Tww)
                        ews2 = wsp.tile([2, TWMAX * P], BF, tag="ews2",
                                        name="ewsf")
                        nc.sync.dma_start(
                            out=ews2[0:2, :Tww * P],
                            in_=ew2_d[0:2, base * P:(base + Tww) * P])
                        cht = ps.tile([P, HEADS, P], F32, tag="cht",
                                      name="cht")
                        dnm = ps.tile([P, HEADS], F32, tag="fdnm", name="fdnm")
                        pend = []

                        def emit_cht(t0p, J, col0, lt0p, Bp, wqp, first,
                                     last, STx):
                            for j in range(J):
                                nc.tensor.matmul(
                                    out=cht[:].rearrange("p h c -> p (h c)"),
                                    lhsT=gq[:, col0 + j, :],
                                    rhs=Bp[:, j, :, :].rearrange(
                                        "p h c -> p (h c)"),
                                    start=(first and j == 0),
                                    stop=(last and j == J - 1))
                                nc.tensor.matmul(out=dnm[:],
                                                 lhsT=STx[:, :, lt0p + j],
                                                 rhs=wqp[:, j, :],
                                                 start=(first and j == 0),
                                                 stop=(last and j == J - 1))

                        lastsplit = max(si for si in range(S)
                                        if TS[si, w] > 0)
                        first = True
                        for si in range(S):
                            Th = int(TS[si, w])
                            if Th == 0:
                                continue
                            gq0 = g["smap"][si][w] + int(boff[si])
                            lt0 = int(run_off[si, w])
                            for t0p in range(0, Th, 2):
                                J = min(2, Th - t0p)
                                ea2 = wsp.tile([P, 2, DF], BF, tag="fea",
                                               bufs=3, name="fea")
                                for j in range(J):
                                    lt = lt0 + t0p + j
                                    col = gq0 + t0p + j
                                    gp = ps.tile([P, 2, P], BF, tag="pt",
                                                 bufs=2, name="gp")
                                    nc.tensor.transpose(out=gp[:, 0, :],
                                                        in_=gq[:, col, :],
                                                        identity=identb[:])
                                    nc.tensor.transpose(out=gp[:, 1, :],
                                                        in_=ST[:, :, lt],
                                                        identity=identb[:])
                                    gS = wsp.tile([P, 2, P], BF, tag="ghT",
                                                  bufs=3, name="ghT")
                                    nc.scalar.activation(out=gS[:],
                                                         in_=gp[:],
                                                         func=AF.Identity)
                                    ep = ps.tile([P, DF], F32, tag="ep",
                                                 bufs=3, name="fep")
                                    nc.tensor.matmul(
                                        out=ep[:],
                                        lhsT=ews2[0:2, lt * P:(lt + 1) * P],
                                        rhs=fwebb_s[0:2, :],
                                        start=True, stop=False)
                                    nc.tensor.matmul(out=ep[:],
                                                     lhsT=gS[:, 0, :],
                                                     rhs=wlf, start=False,
                                                     stop=False)
                                    nc.tensor.matmul(out=ep[:],
                                                     lhsT=gS[:, 1, :],
                                                     rhs=xrf[:], start=False,
                                                     stop=True)
                                    leaky(ea2[:, j, :], ep[:], [P, DF])
                                lg2 = wsp.tile([P, 2, DF], BF, tag="flg",
                                               bufs=3, name="flg")
                                nc.vector.tensor_tensor(
                                    out=lg2[:, :J, :], in0=ea2[:, :J, :],
                                    in1=attf_s[:, None, :].to_broadcast(
                                        [P, J, DF]),
                                    op=OP.mult)
                                lgr2 = wsp.tile([P, 2 * HEADS], BF,
                                                tag="flgr", name="flgr")
                                with nc.allow_low_precision(
                                        reason="bf16 head-logit reduce"):
                                    nc.vector.tensor_reduce(
                                        out=lgr2[:, :J * HEADS].rearrange(
                                            "p (j h) -> p j h", h=HEADS),
                                        in_=lg2[:, :J, :].rearrange(
                                            "p j (h c) -> p j h c", h=HEADS,
                                            c=P),
                                        axis=AX.X, op=OP.add)
                                wqp = wsp.tile([P, 2, HEADS], BF, tag="fwq",
                                               bufs=3, name="fwq")
                                nc.scalar.activation(
                                    out=wqp[:, :J, :],
                                    in_=lgr2[:, :J * HEADS].rearrange(
                                        "p (j h) -> p j h", h=HEADS),
                                    func=AF.Exp)
                                Bp = wsp.tile([P, 2, HEADS, P], BF, tag="fB",
                                              bufs=3, name="fB")
                                nc.vector.tensor_tensor(
                                    out=Bp[:, :J, :, :],
                                    in0=ST[:, :, lt0 + t0p:lt0 + t0p + J]
                                        .rearrange("p d j -> p j d")
                                        [:, :, None, :]
                                        .to_broadcast([P, J, HEADS, P]),
                                    in1=wqp[:, :J, :, None].to_broadcast(
                                        [P, J, HEADS, P]),
                                    op=OP.mult)
                                last = (si == lastsplit and
                                        t0p + J == Th)
                                if pend:
                                    emit_cht(*pend.pop())
                                pend.append((t0p, J, gq0 + t0p, lt0 + t0p,
                                             Bp, wqp, first, last, ST))
                                first = False
                        for args in pend:
                            emit_cht(*args)

                        # flush: nmr_h = ChT_h^T @ Wlf_h; out = bias +
                        #        mean_h numer/denom
                        chsb = wsp.tile([P, HEADS, P], BF, tag="chsb",
                                        name="chsb")
                        nc.scalar.activation(out=chsb[:], in_=cht[:],
                                             func=AF.Identity)
                        nmr = ps.tile([P, DF], F32, tag="ep", bufs=3,
                                      name="fnmr")
                        for h in range(HEADS):
                            nc.tensor.matmul(
                                out=nmr[:, h * P:(h + 1) * P],
                                lhsT=chsb[:, h, :],
                                rhs=wlf[:, h * P:(h + 1) * P],
                                start=True, stop=True)
                        rd = wsp.tile([P, HEADS], F32, tag="rd", name="frd")
                        nc.vector.tensor_scalar(out=rd[:], in0=dnm[:],
                                                scalar1=1e-16, scalar2=None,
                                                op0=OP.add)
                        nc.vector.reciprocal(out=rd[:], in_=rd[:])
                        nc.vector.tensor_scalar(out=rd[:], in0=rd[:],
                                                scalar1=1.0 / HEADS,
                                                scalar2=None, op0=OP.mult)
                        sc = wsp.tile([P, HEADS, P], F32, tag="sc", bufs=1,
                                      name="sc")
                        nc.vector.tensor_tensor(
                            out=sc[:],
                            in0=nmr[:].rearrange("p (h c) -> p h c", h=HEADS,
                                                 c=P),
                            in1=rd[:, :, None].to_broadcast([P, HEADS, P]),
                            op=OP.mult)
                        acc = wsp.tile([P, P], F32, tag="acc", name="acc")
                        nc.vector.tensor_reduce(
                            out=acc[:], in_=sc[:].rearrange("p h c -> p c h"),
                            axis=AX.X, op=OP.add)
                        nc.vector.tensor_tensor(out=acc[:], in0=acc[:],
                                                in1=fpk_s[:], op=OP.add)
                        nc.sync.dma_start(out=out_d[w * P:(w + 1) * P, :],
                                          in_=acc[:])

        # ---- the 4 layers (layer li+1's staging fused into li's P3) ----
        with tc.tile_pool(name="psS", bufs=1, space="PSUM") as psS:
            st0, stage0 = make_stager(0)
            for w in range(NW):
                stage0(w, psS)
        xr = st0["xr_all"]
        xr = hidden_layer(0, False, xr, 1)
        xr = hidden_layer(1, True, xr, 2)
        hidden_layer(2, False, xr, 3)
        final_layer()

    nc.compile()
    return nc


# ----------------------------------------------------------------------------
# Host-side driver
# ----------------------------------------------------------------------------

def _repP(v):
    v = np.asarray(v, np.float32).reshape(-1)
    return np.broadcast_to(v, (P, v.shape[0]))


def _winmaj(arr, lo_c, hi_c, NW):
    """[n, P] node-major slice -> [P, NW*P] window-major (padded)."""
    out = np.zeros((NW * P, P), np.float32)
    out[:hi_c - lo_c] = arr[lo_c:hi_c]
    return np.ascontiguousarray(
        out.reshape(NW, P, P).transpose(1, 0, 2).reshape(P, NW * P))


def make_in_maps(meta, inputs):
    NW, TT = meta["NW"], meta["TT"]
    lo, hi = meta["lo"], meta["hi"]
    x = np.asarray(inputs["x"], np.float32)
    resid = np.asarray(inputs["residual"], np.float32)
    ew = np.asarray(inputs["edge_weight"], np.float32)

    att = np.asarray(inputs["att"], np.float32)      # (3, H, C)
    attf = np.asarray(inputs["att_f"], np.float32)   # (H, DOUT)
    bl = np.asarray(inputs["bl"], np.float32)
    br = np.asarray(inputs["br"], np.float32)
    bias = np.asarray(inputs["bias"], np.float32)
    blf = np.asarray(inputs["bl_f"], np.float32)
    brf = np.asarray(inputs["br_f"], np.float32)
    biasf = np.asarray(inputs["bias_f"], np.float32)

    bf16 = ml_dtypes.bfloat16
    # hidden features stored (c h)-interleaved so the DVE alpha-weighting
    # multiply has a packed last dim (2x mode); PRM[c*H+h] = h*CH+c
    PRM = np.array([h * CH + c for c in range(CH) for h in range(HEADS)])
    wpk = np.concatenate(
        [np.asarray(inputs["Wl"], np.float32)[i][:, PRM] for i in range(3)]
        + [np.asarray(inputs["Wr"], np.float32)[i][:, PRM] for i in range(3)]
        + [np.asarray(inputs["Wl_f"], np.float32),
           np.asarray(inputs["Wr_f"], np.float32)], axis=1).astype(bf16)
    cpk = np.concatenate(
        [_repP(inputs["ln_w"][i]) for i in range(3)]
        + [_repP(inputs["ln_b"][i]) for i in range(3)]
        + [_repP(bias[i] + bl[i]) for i in range(3)], axis=1).astype(np.float32)
    apk = np.concatenate([_repP(att[i].reshape(-1)[PRM]) for i in range(3)],
                         axis=1).astype(bf16)
    We = np.asarray(inputs["We"], np.float32)
    webb = np.zeros((5, 12 * P), np.float32)
    for l in range(3):
        webb[0, l * 4 * P:(l + 1) * 4 * P] = np.tile((bl[l] + br[l])[PRM], 4)
        for r in range(4):
            webb[1 + r, l * 4 * P + r * P:l * 4 * P + (r + 1) * P] = \
                We[l].reshape(P)[PRM]
    fwebb = np.stack([blf + brf,
                      np.asarray(inputs["We_f"], np.float32).reshape(DF)])
    attf_rep = _repP(attf).astype(bf16)
    biaf_eff = biasf + blf.reshape(HEADS, -1).mean(axis=0)
    fpk = _repP(biaf_eff).astype(np.float32)

    common = dict(invd=meta["invd"].astype(np.float32), cpk=cpk, apk=apk,
                  wpk=wpk, attf=attf_rep, fpk=fpk,
                  webb=webb.astype(bf16), fwebb=fwebb.astype(bf16))

    S = meta["S"]
    woff, TS, run_off = meta["woff"], meta["TS"], meta["run_off"]
    nqcum, qoff, NQTOT = meta["nqcum"], meta["qoff"], meta["NQTOT"]

    in_maps = []
    for c in range(NCORES):
        ewc = np.zeros(TT * P, np.float32)
        m = meta["ewsl"][c] >= 0
        ewc[m] = ew[meta["ewsl"][c][m]]
        ew2 = np.stack([np.ones(TT * P, np.float32), ewc])
        ew5 = np.zeros((5, NQTOT * P), np.float32)
        ew5[0] = 1.0
        for w in range(NW):
            for si in range(S):
                Th = int(TS[si, w])
                lt0 = int(run_off[si, w])
                qq0 = int(qoff[w]) + int(nqcum[si, w])
                for q0 in range(0, Th, 4):
                    qq = qq0 + q0 // 4
                    for r in range(min(4, Th - q0)):
                        gt = int(woff[w]) + lt0 + q0 + r
                        ew5[1 + r, qq * P:(qq + 1) * P] = \
                            ewc[gt * P:(gt + 1) * P]
        in_maps.append(dict(
            h0s=_winmaj(x, lo[c], hi[c], NW).astype(bf16),
            rs=_winmaj(resid, lo[c], hi[c], NW).astype(bf16),
            dsti=meta["dsti"][c].astype(bf16),
            idx=np.tile(meta["idxm"][c], (8, 1)),
            ew5=ew5.astype(bf16),
            ew2=ew2.astype(bf16),
            gmat=meta["gmat"][c].astype(bf16),
            gmatT=meta["gmatT"][c].astype(bf16),
            **common))
    return in_maps


def assemble(meta, results):
    N = meta["N"]
    lo, hi = meta["lo"], meta["hi"]
    out = np.zeros((N, P), np.float32)
    for c in range(NCORES):
        n = int(hi[c] - lo[c])
        out[lo[c]:hi[c]] = results[c]["out"][:n]
    return out


_CACHE = {}


def kernel(**inputs):
    ei = np.asarray(inputs["edge_index"])
    bt = np.asarray(inputs["batch"])
    key = (ei.shape, bt.shape, hash(ei.tobytes()), hash(bt.tobytes()))
    if key not in _CACHE:
        meta = build_meta(ei, bt)
        nc = build_program(meta)
        _CACHE[key] = (meta, nc)
    meta, nc = _CACHE[key]
    in_maps = make_in_maps(meta, inputs)
    res = run_bass_kernel_spmd(nc, in_maps, list(range(NCORES)))
    return assemble(meta, res.results)

